# revision 27
# baseline (speedup 1.0000x reference)
"""Causal single-head attention layer on 8 TRN2 NeuronCores.

Reference (per batch b):
  Q = x@Wq+bq; K = x@Wk+bk; V = x@Wv+bv        (S=4096, D=512, H=64)
  S = Q K^T / sqrt(S);  P = softmax(S + causal_mask);  out = (P V) @ Wo + bo

Sharding: 8 cores = 4 batches x 2 "halves". Each core owns 4 query-blocks
of 512 rows of its batch: even cores take blocks [7,4,3,0], odd take
[6,5,2,1] (causal work 72 k-tiles each). SPMD requires one program, so
both core types run the same *structural* schedule with per-slot k-tile
counts NKT=[32,24,16,8]; over-structural/diagonal k-tiles are killed by
per-core mask data: slots 0-2 add shipped 0/-2048 bias tiles into the
scores on the PE (identity matmul), slot 3 (processed first, while the
PE pipeline is still filling) multiplies P by an on-chip ramp>=threshold
compare on the vector engine. No collectives are needed.

DMA strategy: a dma_start costs ~2us fixed + bytes/436GB/s and transfers
serialize per queue, and HWDGE (sync+scalar) completion semaphores
round-robin 8 shared lanes — so ship FEW, LARGE, host-packed 2D
transfers: one weight pack + x^T halves on the two HWDGE rings (8 HWDGE
DMAs total = no lane aliasing), xtq/bias-tiles/K^T-repacks on the gpsimd
SWDGE ring, and the output staged wide and shipped once per slot into a
host-unscrambled layout.

On-chip algorithm per core (all matmuls fp16, fp32 PSUM accumulate):
  xt (D-on-partition x^T, host-pretransposed) -> K^T,V^T proj (stacked
  [Wv|Wk] stationary) and Q^T proj on host-permuted xtq with duplicated
  [Wq|Wq] so Q^T lands on both partition halves.
  K^T is repacked (even k-tiles -> partitions 0:64, odd -> 64:128) so each
  S^T pair runs as two CONCURRENT PE row-tile matmuls (tile_position (0,0)
  and (64,0)), doubling S^T throughput.
  V^T -> V via PE transposes; V gets a ones column appended so the softmax
  denominator falls out of the AV matmul for free.
  Per group g: S^T [128k x 1024q] (+ masking) -> exp (ACT, scale 1/64) ->
  fp16 P -> AV accumulate out^T_aug [65, 512].
  Final: y = (out^T_aug.T @ [Wo; bv@Wo+bo]) * (1/denom).
  Softmax max-subtraction skipped: |S/64| <~ 1 so exp is safe.
  Slots are processed smallest-k-range first ([3,2,1,0]) so production
  stays ahead; emission is software-pipelined (AV lags S^T by 2 groups,
  projections interleaved, epilogues split in halves).
"""

import os
import math

os.environ.setdefault("MYCRO_LOCAL_CACHE", "1")

import numpy as np

import concourse.bass as bass
import concourse.mybir as mybir
import concourse.tile as tile
from concourse import bacc
from concourse.bass_utils import run_bass_kernel_spmd
from concourse.masks import make_identity

F32 = mybir.dt.float32
F16 = mybir.dt.float16
I16 = mybir.dt.int16

B, S, D, H = 4, 4096, 512, 64
QB = 512                  # query block
NKT = [32, 24, 16, 8]     # structural k-tiles (of 128) per slot
BLOCKS_EVEN = [7, 4, 3, 0]
BLOCKS_ODD = [6, 5, 2, 1]
NGRP = [n // 2 for n in NKT]          # groups (pairs of k-tiles) per slot
SLOT_ORDER = [3, 2, 1, 0]             # smallest k-range first
QPOS = {s: i for i, s in enumerate(SLOT_ORDER)}   # xtq column block of slot
NEG_BIAS = -2048.0                    # exp(-2048/64) == 0
N_DUMMY = 9                           # PE HAM warm-up matmuls

LAST_EXEC_TIME_NS = None
LAST_RESULTS = None


def _install_ntff_hook():
    """Register the axon NTFF profile hook if the image's antenv lacks it,
    so run_bass_kernel_spmd(trace=True) can report real exec_time_ns."""
    import sys
    import types
    try:
        from antenv.axon_hooks import get_axon_ntff_profile_hook  # noqa: F401
        return True  # already present
    except ImportError:
        pass
    try:
        import trn_agent_boot.trn_boot as _tb
        hook = _tb._ntff_profile_via_ctypes("/opt/axon/libaxon_pjrt.so")
        if hook is None:
            return False
        mod = types.ModuleType("antenv.axon_hooks")
        mod.get_axon_ntff_profile_hook = lambda: hook
        mod.set_axon_ntff_profile_hook = lambda h: None
        sys.modules["antenv.axon_hooks"] = mod
        return True
    except Exception:
        return False


def _build_nc():
    nc = bacc.Bacc(
        "TRN2",
        target_bir_lowering=False,
        debug=False,
        enable_asserts=False,
        num_devices=8,
    )

    # host-packed inputs (see _make_in_maps for the layouts)
    xtb_d = nc.dram_tensor("xtb", [128, 16384], F16, kind="ExternalInput")
    xtqb_d = nc.dram_tensor("xtqb", [128, 8192], F16, kind="ExternalInput")
    wpack_d = nc.dram_tensor("wpack", [128, 1536], F16, kind="ExternalInput")
    b2t_d = nc.dram_tensor("b2t", [128, 18], F32, kind="ExternalInput")
    biasm_d = nc.dram_tensor("biasm", [128, 12 * 1024], F16, kind="ExternalInput")
    out_d = nc.dram_tensor("out", [128, 8192], F16, kind="ExternalOutput")

    krepeat = int(os.environ.get("KREPEAT", "1"))
    with tile.TileContext(nc) as tc:
      for _rep in range(krepeat):
        with (
            tc.tile_pool(name="big", bufs=1) as big,
            tc.tile_pool(name="small", bufs=1) as small,
        ):
            # ---- persistent SBUF tensors ----
            xtb_sb = big.tile([128, 16384], F16, tag="xtb")
            xtqb_sb = big.tile([128, 8192], F16, tag="xtqb")
            kvt_sb = big.tile([128, S], F16, tag="kvt")     # 0:64 V^T, 64:128 K^T
            ktp_sb = big.tile([128, S // 2], F16, tag="ktp")  # packed K^T even|odd
            qtp_sb = big.tile([128, 4 * QB], F16, tag="qtp")  # Q^T dup halves
            vaug_sb = big.tile([128, 32 * 80], F16, tag="vaug")
            biasm_sb = big.tile([128, 12 * 1024], F16, tag="biasm")
            wpack_sb = small.tile([128, 1536], F16, tag="wpack")
            b2t_sb = small.tile([128, 18], F32, tag="b2t")
            ident_sb = small.tile([64, 64], F16, tag="ident")
            id128_sb = small.tile([128, 128], F16, tag="id128")
            ones_sb = small.tile([1, 1], F16, tag="ones")
            r2i_sb = small.tile([128, 1024], I16, tag="r2i")
            r2_sb = small.tile([128, 1024], F16, tag="r2")
            dummy_sb = small.tile([64, 512], F16, tag="dummy")
            warm_sb = small.tile([1, 2], F32, tag="warm")

            # ---- input DMAs: few, large, host-packed ----
            # xtb is block-major (col = sb*2048 + j*512 + c): the first two
            # 512KB transfers unblock KV blocks 0/1 as early as possible,
            # the rest rides in bigger chunks
            nc.scalar.dma_start(out=xtqb_sb[:, 0:4096], in_=xtqb_d[:, 0:4096])
            nc.scalar.dma_start(out=wpack_sb[:], in_=wpack_d[:, :])
            nc.scalar.dma_start(out=b2t_sb[:], in_=b2t_d[:, :])
            nc.scalar.dma_start(out=xtb_sb[:, 8192:12288],
                                in_=xtb_d[:, 8192:12288])
            nc.scalar.dma_start(out=xtb_sb[:, 12288:16384],
                                in_=xtb_d[:, 12288:16384])
            for idx0 in (8, 4, 0):
                nc.scalar.dma_start(
                    out=biasm_sb[:, idx0 * 1024:(idx0 + 4) * 1024],
                    in_=biasm_d[:, idx0 * 1024:(idx0 + 4) * 1024],
                )
            nc.sync.dma_start(out=xtb_sb[:, 0:4096], in_=xtb_d[:, 0:4096])
            nc.sync.dma_start(out=xtb_sb[:, 4096:8192], in_=xtb_d[:, 4096:8192])

            # ---- on-chip constants ----
            nc.vector.memset(dummy_sb[:], 0.0)
            nc.vector.memset(warm_sb[:, 0:1], 0.0)
            # preload the ACT exp table set before the real activations
            nc.scalar.activation(
                warm_sb[:, 1:2], warm_sb[:, 0:1],
                mybir.ActivationFunctionType.Exp,
            )
            make_identity(nc, ident_sb[:])
            make_identity(nc, id128_sb[:])
            nc.vector.memset(ones_sb[:], 1.0)
            # ramp R2[p, u*512+f] = f - p - 128*u  (slot-3 causal masking)
            nc.gpsimd.iota(
                r2i_sb[:], pattern=[[-128, 2], [1, 512]], base=0,
                channel_multiplier=-1,
            )
            nc.vector.tensor_copy(r2_sb[:], r2i_sb[:])
            vaug3 = vaug_sb[:].rearrange("p (k c) -> p k c", c=80)
            nc.vector.memset(vaug3[:, :, 64:65], 1.0)

            def emit_xtq_dma(h):
                # xtq is host-packed in slot processing order; half h=0
                # covers slots [3,2], h=1 covers [1,0]
                nc.gpsimd.dma_start(
                    out=xtqb_sb[:, h * 4096:(h + 1) * 4096],
                    in_=xtqb_d[:, h * 4096:(h + 1) * 4096],
                )

            with (
                tc.tile_pool(name="projps", bufs=2, space="PSUM") as projps,
                tc.tile_pool(name="stps", bufs=2, space="PSUM") as stps,
                tc.tile_pool(name="otps", bufs=1, space="PSUM") as otps,
                tc.tile_pool(name="ptp", bufs=6) as ptp,
                tc.tile_pool(name="epi", bufs=6) as epi,
                tc.tile_pool(name="ysbp", bufs=2) as ysbp,
            ):
                # PE HAM warm-up: dummy matmuls while input DMAs stream
                for _ in range(N_DUMMY):
                    dmy = projps.tile([64, 512], F32, name="pp", tag="pp")
                    nc.tensor.matmul(
                        dmy[:], lhsT=dummy_sb[:, 0:64], rhs=dummy_sb[:],
                        start=True, stop=True,
                    )

                kv_k = kvt_sb[64:128, :].rearrange(
                    "p (g u c) -> p g u c", u=2, c=128
                )

                def emit_KV(sb):
                    # KV projection for column block sb (k-tiles 4sb..4sb+3)
                    kvp = projps.tile([128, 512], F32, name="pp", tag="pp")
                    for j in range(4):
                        c0 = sb * 2048 + j * 512
                        nc.tensor.matmul(
                            kvp[:],
                            lhsT=wpack_sb[:, j * 128:(j + 1) * 128],
                            rhs=xtb_sb[:, c0:c0 + 512],
                            start=(j == 0),
                            stop=(j == 3),
                        )
                    nc.vector.tensor_scalar_add(
                        kvt_sb[:, sb * 512:(sb + 1) * 512], kvp[:],
                        b2t_sb[:, 0:1],
                    )

                def emit_VTPK(sb):
                    # repack K^T: even k-tiles -> partitions 0:64, odd -> 64:128
                    nc.gpsimd.dma_start(
                        out=ktp_sb[0:64, sb * 256:(sb + 1) * 256],
                        in_=kv_k[:, 2 * sb:2 * sb + 2, 0:1, :],
                    )
                    nc.gpsimd.dma_start(
                        out=ktp_sb[64:128, sb * 256:(sb + 1) * 256],
                        in_=kv_k[:, 2 * sb:2 * sb + 2, 1:2, :],
                    )
                    # V natural tiles via PE transpose
                    for kt in range(4 * sb, 4 * sb + 4):
                        vtp = projps.tile([128, 64], F16, name="pp", tag="pp")
                        nc.tensor.transpose(
                            vtp[:], kvt_sb[0:64, kt * 128:(kt + 1) * 128],
                            ident_sb[:],
                        )
                        nc.vector.tensor_copy(
                            vaug_sb[:, kt * 80:kt * 80 + 64], vtp[:]
                        )

                def emit_P(sb):
                    emit_KV(sb)
                    emit_VTPK(sb)

                def emit_Q(s):
                    hq, q2 = QPOS[s] // 2, QPOS[s] % 2
                    qp = projps.tile([128, 512], F32, name="pp", tag="pp")
                    for j in range(4):
                        c0 = hq * 4096 + j * 1024 + q2 * 512
                        nc.tensor.matmul(
                            qp[:],
                            lhsT=wpack_sb[:, 512 + j * 128:512 + (j + 1) * 128],
                            rhs=xtqb_sb[:, c0:c0 + 512],
                            start=(j == 0),
                            stop=(j == 3),
                        )
                    nc.vector.tensor_scalar_add(
                        qtp_sb[:, s * 512:(s + 1) * 512], qp[:],
                        b2t_sb[:, 1:2],
                    )

                groups = [(s, g) for s in SLOT_ORDER for g in range(NGRP[s])]
                otp_of = {}
                pt_of = {}

                def emit_S(i):
                    s, g = groups[i]
                    if g == 0:
                        otp_of[s] = (
                            otps.tile([H + 1, 512], F32, name="otpA", tag="otpA"),
                            otps.tile([H + 1, 512], F32, name="otpB", tag="otpB"),
                        )
                    masked = g >= NGRP[s] - 4
                    pe_mask = masked and s != 3
                    stp = stps.tile([128, 1024], F32, name="stp", tag="stp")
                    if s == 3:
                        # first slot: skip the K^T repack dependency, read
                        # K^T straight from kvt (both tiles on row group 64)
                        for u in range(2):
                            kt = 2 * g + u
                            nc.tensor.matmul(
                                stp[:, u * 512:(u + 1) * 512],
                                lhsT=kvt_sb[64:128, kt * 128:(kt + 1) * 128],
                                rhs=qtp_sb[64:128, s * 512:(s + 1) * 512],
                                start=True, stop=True,
                                tile_position=(64, 0),
                            )
                    else:
                        nc.tensor.matmul(
                            stp[:, 0:512],
                            lhsT=ktp_sb[0:64, g * 128:(g + 1) * 128],
                            rhs=qtp_sb[0:64, s * 512:(s + 1) * 512],
                            start=True, stop=not pe_mask,
                            tile_position=(0, 0),
                        )
                        nc.tensor.matmul(
                            stp[:, 512:1024],
                            lhsT=ktp_sb[64:128, g * 128:(g + 1) * 128],
                            rhs=qtp_sb[64:128, s * 512:(s + 1) * 512],
                            start=True, stop=not pe_mask,
                            tile_position=(64, 0),
                        )
                    if pe_mask:
                        # add 0/-2048 bias tiles into the scores on the PE
                        idx = s * 4 + (g - (NGRP[s] - 4))
                        for u in range(2):
                            nc.tensor.matmul(
                                stp[:, u * 512:(u + 1) * 512],
                                lhsT=id128_sb[:],
                                rhs=biasm_sb[:, idx * 1024 + u * 512:
                                             idx * 1024 + (u + 1) * 512],
                                start=False, stop=True,
                            )
                    pt = ptp.tile([128, 1024], F16, name="pt", tag="pt")
                    nc.scalar.activation(
                        pt[:], stp[:], mybir.ActivationFunctionType.Exp,
                        scale=1.0 / 64.0,
                    )
                    if masked and s == 3:
                        # slot 3 (pipeline-fill phase): mask P on the DVE
                        nc.vector.scalar_tensor_tensor(
                            pt[:], r2_sb[:], b2t_sb[:, 14 + g:15 + g], pt[:],
                            op0=mybir.AluOpType.is_ge,
                            op1=mybir.AluOpType.mult,
                        )
                    pt_of[i] = pt

                def emit_AV(i):
                    # even k-tiles accumulate into otpA, odd into otpB, so
                    # consecutive matmuls hit different PSUM banks and their
                    # fill/drain phases overlap
                    s, g = groups[i]
                    pt = pt_of.pop(i)
                    for u in range(2):
                        kt = 2 * g + u
                        nc.tensor.matmul(
                            otp_of[s][u][:],
                            lhsT=vaug_sb[:, kt * 80:kt * 80 + 65],
                            rhs=pt[:, u * 512:(u + 1) * 512],
                            start=(kt == u),
                            stop=(kt == NKT[s] - 2 + u),
                        )

                epi_st = {}

                def emit_E_half(s, half):
                    last = s == SLOT_ORDER[-1]
                    if half == 0:
                        otpA, otpB = otp_of.pop(s)
                        ot16 = epi.tile([H + 1, 512], F16, name="ot16", tag="ot16")
                        dnrow = epi.tile([1, 512], F16, name="dnrow", tag="dnrow")
                        nc.vector.tensor_copy(ot16[:], otpA[:])
                        nc.vector.tensor_add(ot16[:], ot16[:], otpB[:])
                        nc.vector.tensor_copy(dnrow[:], ot16[64:65, :])
                        ysbw = ysbp.tile([128, 2048], F16, name="ysbw", tag="ysbw")
                        # all four denominator transposes into one PSUM tile,
                        # one batched reciprocal
                        dnp = projps.tile([128, 4], F32, name="pp", tag="pp")
                        for t in range(4):
                            nc.tensor.matmul(
                                dnp[:, t:t + 1],
                                lhsT=dnrow[:, t * 128:(t + 1) * 128],
                                rhs=ones_sb[:],
                                start=True, stop=True,
                            )
                        recip = epi.tile([128, 4], F32, name="recip", tag="recip")
                        nc.vector.reciprocal(recip[:], dnp[:])
                        epi_st[s] = (ot16, dnrow, ysbw, recip)
                    ot16, dnrow, ysbw, recip = epi_st[s]
                    for t in (0, 1) if half == 0 else (2, 3):
                        yp = projps.tile([128, 512], F32, name="pp", tag="pp")
                        nc.tensor.matmul(
                            yp[:],
                            lhsT=ot16[:, t * 128:(t + 1) * 128],
                            rhs=wpack_sb[0:65, 1024:1536],
                            start=True, stop=True,
                        )
                        if last and t in (1, 3):
                            # scalar engine is done with exps by now: use its
                            # free affine to scale, halving the tail chain
                            nc.scalar.activation(
                                ysbw[:, t * 512:(t + 1) * 512], yp[:],
                                mybir.ActivationFunctionType.Copy,
                                scale=recip[:, t:t + 1],
                            )
                        else:
                            nc.vector.tensor_scalar_mul(
                                ysbw[:, t * 512:(t + 1) * 512], yp[:],
                                recip[:, t:t + 1],
                            )
                        if last and t in (1, 3):
                            nc.sync.dma_start(
                                out=out_d[:, s * 2048 + (t - 1) * 512:
                                          s * 2048 + (t + 1) * 512],
                                in_=ysbw[:, (t - 1) * 512:(t + 1) * 512],
                            )
                    if half == 1 and not last:
                        nc.sync.dma_start(
                            out=out_d[:, s * 2048:(s + 1) * 2048],
                            in_=ysbw[:],
                        )

                # ---- software-pipelined emission ----
                prod = {
                    0: [lambda: emit_Q(3), lambda: emit_P(0)],
                    2: [lambda: emit_P(1), lambda: emit_Q(2),
                        lambda: emit_xtq_dma(1)],
                    4: [lambda: emit_P(2)],
                    8: [lambda: emit_P(3)],
                    12: [lambda: emit_Q(1), lambda: emit_P(4)],
                    16: [lambda: emit_P(5)],
                    20: [lambda: emit_P(6)],
                    24: [lambda: emit_Q(0)],
                    28: [lambda: emit_P(7)],
                }
                last_step_of_slot = {}
                acc = -1
                for s in SLOT_ORDER:
                    acc += NGRP[s]
                    last_step_of_slot[s] = acc

                n = len(groups)
                for i in range(n + 4):
                    for fn in prod.get(i, []):
                        fn()
                    if i < n:
                        emit_S(i)
                    if 0 <= i - 3 < n:
                        emit_AV(i - 3)
                        for s in SLOT_ORDER:
                            if last_step_of_slot[s] == i - 3:
                                emit_E_half(s, 0)
                    if 0 <= i - 4 < n:
                        for s in SLOT_ORDER:
                            if last_step_of_slot[s] == i - 4:
                                emit_E_half(s, 1)

    nc.compile()
    return nc


_NC_CACHE = {}


def _thr_row(blocks):
    # mask P[k_local, u*512+f] iff  f - p - 128*u < thr[s, j]
    # thr = 128*t0 - 512*block  with t0 = NKT[s]-8+2j  (even tile of group)
    t = np.zeros(16, np.float32)
    for s in range(4):
        for j in range(4):
            t0 = NKT[s] - 8 + 2 * j
            t[s * 4 + j] = 128.0 * t0 - 512.0 * blocks[s]
    return t


def _bias_tiles(blocks):
    # bias[p, idx*1024 + u*512+f] = NEG_BIAS where masked (slots 0-2 only)
    p = np.arange(128)[:, None, None]
    cols = np.arange(1024)[None, None, :]
    r2 = (cols % 512) - p - 128 * (cols // 512)
    thr = _thr_row(blocks)[:12].reshape(1, 12, 1)
    bias = np.where(r2 < thr, np.float32(NEG_BIAS), np.float32(0.0))
    return bias.reshape(128, 12 * 1024).astype(np.float16)


def _make_in_maps(x, Wq, bq, Wk, bk, Wv, bv, Wo, bo):
    # weight pack: [wkv(j-chunked) | wq2(j-chunked) | wo_aug padded]
    wkv = np.concatenate([Wv, Wk], axis=1).astype(np.float16)
    wkv = wkv.reshape(4, 128, 128).transpose(1, 0, 2).reshape(128, 512)
    wq2 = np.concatenate([Wq, Wq], axis=1).astype(np.float16)
    wq2 = wq2.reshape(4, 128, 128).transpose(1, 0, 2).reshape(128, 512)
    wo_aug = np.concatenate([Wo, (bv @ Wo + bo)[None, :]], axis=0)
    wop = np.zeros((128, 512), np.float16)
    wop[0:65] = wo_aug.astype(np.float16)
    wpack = np.ascontiguousarray(
        np.concatenate([wkv, wq2, wop], axis=1))           # (128, 1536)

    # biases + slot-3 thresholds: [bkv | bq2 | thr(16)]
    bkv = np.concatenate([np.zeros(64, np.float32), bk])
    bq2 = np.concatenate([bq, bq])
    thr_e, thr_o = _thr_row(BLOCKS_EVEN), _thr_row(BLOCKS_ODD)
    b2t_e = np.concatenate(
        [bkv[:, None], bq2[:, None], np.tile(thr_e[None, :], (128, 1))],
        axis=1).astype(np.float32)
    b2t_o = np.concatenate(
        [bkv[:, None], bq2[:, None], np.tile(thr_o[None, :], (128, 1))],
        axis=1).astype(np.float32)

    biasm_even = _bias_tiles(BLOCKS_EVEN)
    biasm_odd = _bias_tiles(BLOCKS_ODD)

    in_maps = []
    for c in range(8):
        b = c // 2
        blocks = BLOCKS_EVEN if c % 2 == 0 else BLOCKS_ODD
        xt = np.ascontiguousarray(x[b].T).astype(np.float16)      # (512, 4096)
        # xtb[p, sb*2048 + j*512 + cc] = xt[j*128+p, sb*512+cc]
        xtb = np.ascontiguousarray(
            xt.reshape(4, 128, 8, 512).transpose(1, 2, 0, 3).reshape(128, 16384))
        qcols = np.concatenate(
            [np.arange(blocks[s] * QB, (blocks[s] + 1) * QB) for s in SLOT_ORDER]
        )
        xtq = xt[:, qcols]                                        # (512, 2048)
        # xtqb[p, H*4096 + j*1024 + q*512 + cc] = xtq[j*128+p, (2H+q)*512+cc]
        xtqb = np.ascontiguousarray(
            xtq.reshape(4, 128, 2, 2, 512).transpose(1, 2, 0, 3, 4).reshape(128, 8192))
        in_maps.append({
            "xtb": xtb,
            "xtqb": xtqb,
            "wpack": wpack,
            "b2t": b2t_e if c % 2 == 0 else b2t_o,
            "biasm": biasm_even if c % 2 == 0 else biasm_odd,
        })
    return in_maps


def kernel(x, Wq, bq, Wk, bk, Wv, bv, Wo, bo):
    global LAST_EXEC_TIME_NS, LAST_RESULTS
    x = np.asarray(x, dtype=np.float32)
    Wq, bq = np.asarray(Wq, np.float32), np.asarray(bq, np.float32)
    Wk, bk = np.asarray(Wk, np.float32), np.asarray(bk, np.float32)
    Wv, bv = np.asarray(Wv, np.float32), np.asarray(bv, np.float32)
    Wo, bo = np.asarray(Wo, np.float32), np.asarray(bo, np.float32)

    if "nc" not in _NC_CACHE:
        _NC_CACHE["nc"] = _build_nc()
    nc = _NC_CACHE["nc"]

    in_maps = _make_in_maps(x, Wq, bq, Wk, bk, Wv, bv, Wo, bo)

    trace = os.environ.get("KERNEL_TRACE", "1") == "1"
    if trace:
        trace = _install_ntff_hook()
    tmpdir = os.environ.get("KERNEL_TRACE_DIR") or None
    try:
        res = run_bass_kernel_spmd(
            nc, in_maps, core_ids=list(range(8)), trace=trace, tmpdir=tmpdir
        )
    except Exception:
        if not trace:
            raise
        res = run_bass_kernel_spmd(nc, in_maps, core_ids=list(range(8)), trace=False)
    LAST_EXEC_TIME_NS = res.exec_time_ns
    LAST_RESULTS = res

    out = np.empty((B, S, D), np.float32)
    for c in range(8):
        b = c // 2
        blocks = BLOCKS_EVEN if c % 2 == 0 else BLOCKS_ODD
        shard2 = np.asarray(res.results[c]["out"], dtype=np.float32)
        # shard2[p, s*2048 + t*512 + cc] = y[slot s][t*128+p, cc]
        y = shard2.reshape(128, 4, 4, 512).transpose(1, 2, 0, 3).reshape(4, 512, 512)
        for s in range(4):
            out[b, blocks[s] * QB:(blocks[s] + 1) * QB, :] = y[s]
    return out


# revision 28
# speedup vs baseline: 1.0758x; 1.0758x over previous
"""Causal single-head attention layer on 8 TRN2 NeuronCores.

Reference (per batch b):
  Q = x@Wq+bq; K = x@Wk+bk; V = x@Wv+bv        (S=4096, D=512, H=64)
  S = Q K^T / sqrt(S);  P = softmax(S + causal_mask);  out = (P V) @ Wo + bo

Sharding: 8 cores = 4 batches x 2 "halves". Each core owns 4 query-blocks
of 512 rows of its batch: even cores take blocks [7,4,3,0], odd take
[6,5,2,1] (causal work 72 k-tiles each). SPMD requires one program, so
both core types run the same *structural* schedule with per-slot k-tile
counts NKT=[32,24,16,8]; over-structural/diagonal k-tiles are killed by
per-core mask data: slots 0-2 add shipped 0/-2048 bias tiles into the
scores on the PE (identity matmul), slot 3 (processed first, while the
PE pipeline is still filling) multiplies P by an on-chip ramp>=threshold
compare on the vector engine. No collectives are needed.

DMA strategy: a dma_start costs ~2us fixed + bytes/436GB/s and transfers
serialize per queue, and HWDGE (sync+scalar) completion semaphores
round-robin 8 shared lanes — so ship FEW, LARGE, host-packed 2D
transfers: one weight pack + x^T halves on the two HWDGE rings (8 HWDGE
DMAs total = no lane aliasing), xtq/bias-tiles/K^T-repacks on the gpsimd
SWDGE ring, and the output staged wide and shipped once per slot into a
host-unscrambled layout.

On-chip algorithm per core (all matmuls fp16, fp32 PSUM accumulate):
  xt (D-on-partition x^T, host-pretransposed) -> K^T,V^T proj (stacked
  [Wv|Wk] stationary) and Q^T proj on host-permuted xtq with duplicated
  [Wq|Wq] so Q^T lands on both partition halves.
  K^T is repacked (even k-tiles -> partitions 0:64, odd -> 64:128) so each
  S^T pair runs as two CONCURRENT PE row-tile matmuls (tile_position (0,0)
  and (64,0)), doubling S^T throughput.
  V^T -> V via PE transposes; V gets a ones column appended so the softmax
  denominator falls out of the AV matmul for free.
  Per group g: S^T [128k x 1024q] (+ masking) -> exp (ACT, scale 1/64) ->
  fp16 P -> AV accumulate out^T_aug [65, 512].
  Final: y = (out^T_aug.T @ [Wo; bv@Wo+bo]) * (1/denom).
  Softmax max-subtraction skipped: |S/64| <~ 1 so exp is safe.
  Slots are processed smallest-k-range first ([3,2,1,0]) so production
  stays ahead; emission is software-pipelined (AV lags S^T by 2 groups,
  projections interleaved, epilogues split in halves).
"""

import os
import math

os.environ.setdefault("MYCRO_LOCAL_CACHE", "1")

import numpy as np

import concourse.bass as bass
import concourse.mybir as mybir
import concourse.tile as tile
from concourse import bacc
from concourse.bass_utils import run_bass_kernel_spmd
from concourse.masks import make_identity

F32 = mybir.dt.float32
F16 = mybir.dt.float16
I16 = mybir.dt.int16

B, S, D, H = 4, 4096, 512, 64
QB = 512                  # query block
NKT = [32, 24, 16, 8]     # structural k-tiles (of 128) per slot
BLOCKS_EVEN = [7, 4, 3, 0]
BLOCKS_ODD = [6, 5, 2, 1]
NGRP = [n // 2 for n in NKT]          # groups (pairs of k-tiles) per slot
SLOT_ORDER = [3, 2, 1, 0]             # smallest k-range first
QPOS = {s: i for i, s in enumerate(SLOT_ORDER)}   # xtq column block of slot
NEG_BIAS = -2048.0                    # exp(-2048/64) == 0
N_DUMMY = 9                           # PE HAM warm-up matmuls

LAST_EXEC_TIME_NS = None
LAST_RESULTS = None


def _install_ntff_hook():
    """Register the axon NTFF profile hook if the image's antenv lacks it,
    so run_bass_kernel_spmd(trace=True) can report real exec_time_ns."""
    import sys
    import types
    try:
        from antenv.axon_hooks import get_axon_ntff_profile_hook  # noqa: F401
        return True  # already present
    except ImportError:
        pass
    try:
        import trn_agent_boot.trn_boot as _tb
        hook = _tb._ntff_profile_via_ctypes("/opt/axon/libaxon_pjrt.so")
        if hook is None:
            return False
        mod = types.ModuleType("antenv.axon_hooks")
        mod.get_axon_ntff_profile_hook = lambda: hook
        mod.set_axon_ntff_profile_hook = lambda h: None
        sys.modules["antenv.axon_hooks"] = mod
        return True
    except Exception:
        return False


def _build_nc():
    nc = bacc.Bacc(
        "TRN2",
        target_bir_lowering=False,
        debug=False,
        enable_asserts=False,
        num_devices=8,
    )

    # host-packed inputs (see _make_in_maps for the layouts)
    xtb_d = nc.dram_tensor("xtb", [128, 16384], F16, kind="ExternalInput")
    xtqb_d = nc.dram_tensor("xtqb", [128, 8192], F16, kind="ExternalInput")
    wpack_d = nc.dram_tensor("wpack", [128, 1536], F16, kind="ExternalInput")
    b2t_d = nc.dram_tensor("b2t", [128, 18], F32, kind="ExternalInput")
    biasm_d = nc.dram_tensor("biasm", [128, 12 * 1024], F16, kind="ExternalInput")
    out_d = nc.dram_tensor("out", [128, 8192], F16, kind="ExternalOutput")

    krepeat = int(os.environ.get("KREPEAT", "1"))
    with tile.TileContext(nc) as tc:
      for _rep in range(krepeat):
        with (
            tc.tile_pool(name="big", bufs=1) as big,
            tc.tile_pool(name="small", bufs=1) as small,
        ):
            # ---- persistent SBUF tensors ----
            xtb_sb = big.tile([128, 16384], F16, tag="xtb")
            xtqb_sb = big.tile([128, 8192], F16, tag="xtqb")
            kvt_sb = big.tile([128, S], F16, tag="kvt")     # 0:64 V^T, 64:128 K^T
            ktp_sb = big.tile([128, S // 2], F16, tag="ktp")  # packed K^T even|odd
            qtp_sb = big.tile([128, 4 * QB], F16, tag="qtp")  # Q^T dup halves
            vaug_sb = big.tile([128, 32 * 80], F16, tag="vaug")
            biasm_sb = big.tile([128, 12 * 1024], F16, tag="biasm")
            wpack_sb = small.tile([128, 1536], F16, tag="wpack")
            b2t_sb = small.tile([128, 18], F32, tag="b2t")
            ident_sb = small.tile([64, 64], F16, tag="ident")
            id128_sb = small.tile([128, 128], F16, tag="id128")
            ones_sb = small.tile([1, 1], F16, tag="ones")
            r2i_sb = small.tile([128, 1024], I16, tag="r2i")
            r2_sb = small.tile([128, 1024], F16, tag="r2")
            dummy_sb = small.tile([64, 512], F16, tag="dummy")
            warm_sb = small.tile([1, 2], F32, tag="warm")

            # ---- input DMAs: few, large, host-packed ----
            # xtb is block-major (col = sb*2048 + j*512 + c): the first two
            # 512KB transfers unblock KV blocks 0/1 as early as possible,
            # the rest rides in bigger chunks
            nc.scalar.dma_start(out=wpack_sb[:], in_=wpack_d[:, :])
            nc.scalar.dma_start(out=b2t_sb[:], in_=b2t_d[:, :])
            for idx0 in (8, 4, 0):
                nc.scalar.dma_start(
                    out=biasm_sb[:, idx0 * 1024:(idx0 + 4) * 1024],
                    in_=biasm_d[:, idx0 * 1024:(idx0 + 4) * 1024],
                )
            nc.scalar.dma_start(out=xtb_sb[:, 8192:12288],
                                in_=xtb_d[:, 8192:12288])
            nc.scalar.dma_start(out=xtb_sb[:, 12288:16384],
                                in_=xtb_d[:, 12288:16384])
            nc.sync.dma_start(out=xtb_sb[:, 0:4096], in_=xtb_d[:, 0:4096])
            nc.sync.dma_start(out=xtb_sb[:, 4096:8192], in_=xtb_d[:, 4096:8192])

            # ---- on-chip constants ----
            nc.vector.memset(dummy_sb[:], 0.0)
            nc.vector.memset(warm_sb[:, 0:1], 0.0)
            # preload the ACT exp table set before the real activations
            nc.scalar.activation(
                warm_sb[:, 1:2], warm_sb[:, 0:1],
                mybir.ActivationFunctionType.Exp,
            )
            make_identity(nc, ident_sb[:])
            make_identity(nc, id128_sb[:])
            nc.vector.memset(ones_sb[:], 1.0)
            # ramp R2[p, u*512+f] = f - p - 128*u  (slot-3 causal masking)
            nc.gpsimd.iota(
                r2i_sb[:], pattern=[[-128, 2], [1, 512]], base=0,
                channel_multiplier=-1,
            )
            nc.vector.tensor_copy(r2_sb[:], r2i_sb[:])
            vaug3 = vaug_sb[:].rearrange("p (k c) -> p k c", c=80)
            nc.vector.memset(vaug3[:, :, 64:65], 1.0)

            def emit_xtq_dma(h):
                # xtq is host-packed in slot processing order; half h=0
                # covers slots [3,2], h=1 covers [1,0]
                nc.gpsimd.dma_start(
                    out=xtqb_sb[:, h * 4096:(h + 1) * 4096],
                    in_=xtqb_d[:, h * 4096:(h + 1) * 4096],
                )

            emit_xtq_dma(0)

            with (
                tc.tile_pool(name="projps", bufs=2, space="PSUM") as projps,
                tc.tile_pool(name="stps", bufs=2, space="PSUM") as stps,
                tc.tile_pool(name="otps", bufs=1, space="PSUM") as otps,
                tc.tile_pool(name="ptp", bufs=6) as ptp,
                tc.tile_pool(name="epi", bufs=6) as epi,
                tc.tile_pool(name="ysbp", bufs=2) as ysbp,
            ):
                # PE HAM warm-up: dummy matmuls while input DMAs stream
                for _ in range(N_DUMMY):
                    dmy = projps.tile([64, 512], F32, name="pp", tag="pp")
                    nc.tensor.matmul(
                        dmy[:], lhsT=dummy_sb[:, 0:64], rhs=dummy_sb[:],
                        start=True, stop=True,
                    )

                kv_k = kvt_sb[64:128, :].rearrange(
                    "p (g u c) -> p g u c", u=2, c=128
                )

                def emit_KV(sb):
                    # KV projection for column block sb (k-tiles 4sb..4sb+3)
                    kvp = projps.tile([128, 512], F32, name="pp", tag="pp")
                    for j in range(4):
                        c0 = sb * 2048 + j * 512
                        nc.tensor.matmul(
                            kvp[:],
                            lhsT=wpack_sb[:, j * 128:(j + 1) * 128],
                            rhs=xtb_sb[:, c0:c0 + 512],
                            start=(j == 0),
                            stop=(j == 3),
                        )
                    nc.vector.tensor_scalar_add(
                        kvt_sb[:, sb * 512:(sb + 1) * 512], kvp[:],
                        b2t_sb[:, 0:1],
                    )

                def emit_VTPK(sb):
                    # repack K^T: even k-tiles -> partitions 0:64, odd -> 64:128
                    nc.gpsimd.dma_start(
                        out=ktp_sb[0:64, sb * 256:(sb + 1) * 256],
                        in_=kv_k[:, 2 * sb:2 * sb + 2, 0:1, :],
                    )
                    nc.gpsimd.dma_start(
                        out=ktp_sb[64:128, sb * 256:(sb + 1) * 256],
                        in_=kv_k[:, 2 * sb:2 * sb + 2, 1:2, :],
                    )
                    # V natural tiles via PE transpose
                    for kt in range(4 * sb, 4 * sb + 4):
                        vtp = projps.tile([128, 64], F16, name="pp", tag="pp")
                        nc.tensor.transpose(
                            vtp[:], kvt_sb[0:64, kt * 128:(kt + 1) * 128],
                            ident_sb[:],
                        )
                        nc.vector.tensor_copy(
                            vaug_sb[:, kt * 80:kt * 80 + 64], vtp[:]
                        )

                def emit_P(sb):
                    emit_KV(sb)
                    emit_VTPK(sb)

                def emit_Q(s):
                    hq, q2 = QPOS[s] // 2, QPOS[s] % 2
                    qp = projps.tile([128, 512], F32, name="pp", tag="pp")
                    for j in range(4):
                        c0 = hq * 4096 + j * 1024 + q2 * 512
                        nc.tensor.matmul(
                            qp[:],
                            lhsT=wpack_sb[:, 512 + j * 128:512 + (j + 1) * 128],
                            rhs=xtqb_sb[:, c0:c0 + 512],
                            start=(j == 0),
                            stop=(j == 3),
                        )
                    nc.vector.tensor_scalar_add(
                        qtp_sb[:, s * 512:(s + 1) * 512], qp[:],
                        b2t_sb[:, 1:2],
                    )

                groups = [(s, g) for s in SLOT_ORDER for g in range(NGRP[s])]
                otp_of = {}
                pt_of = {}

                def emit_S(i):
                    s, g = groups[i]
                    if g == 0:
                        otp_of[s] = (
                            otps.tile([H + 1, 512], F32, name="otpA", tag="otpA"),
                            otps.tile([H + 1, 512], F32, name="otpB", tag="otpB"),
                        )
                    masked = g >= NGRP[s] - 4
                    pe_mask = masked and s != 3
                    stp = stps.tile([128, 1024], F32, name="stp", tag="stp")
                    if s == 3:
                        # first slot: skip the K^T repack dependency, read
                        # K^T straight from kvt (both tiles on row group 64)
                        for u in range(2):
                            kt = 2 * g + u
                            nc.tensor.matmul(
                                stp[:, u * 512:(u + 1) * 512],
                                lhsT=kvt_sb[64:128, kt * 128:(kt + 1) * 128],
                                rhs=qtp_sb[64:128, s * 512:(s + 1) * 512],
                                start=True, stop=True,
                                tile_position=(64, 0),
                            )
                    else:
                        nc.tensor.matmul(
                            stp[:, 0:512],
                            lhsT=ktp_sb[0:64, g * 128:(g + 1) * 128],
                            rhs=qtp_sb[0:64, s * 512:(s + 1) * 512],
                            start=True, stop=not pe_mask,
                            tile_position=(0, 0),
                        )
                        nc.tensor.matmul(
                            stp[:, 512:1024],
                            lhsT=ktp_sb[64:128, g * 128:(g + 1) * 128],
                            rhs=qtp_sb[64:128, s * 512:(s + 1) * 512],
                            start=True, stop=not pe_mask,
                            tile_position=(64, 0),
                        )
                    if pe_mask:
                        # add 0/-2048 bias tiles into the scores on the PE
                        idx = s * 4 + (g - (NGRP[s] - 4))
                        for u in range(2):
                            nc.tensor.matmul(
                                stp[:, u * 512:(u + 1) * 512],
                                lhsT=id128_sb[:],
                                rhs=biasm_sb[:, idx * 1024 + u * 512:
                                             idx * 1024 + (u + 1) * 512],
                                start=False, stop=True,
                            )
                    pt = ptp.tile([128, 1024], F16, name="pt", tag="pt")
                    nc.scalar.activation(
                        pt[:], stp[:], mybir.ActivationFunctionType.Exp,
                        scale=1.0 / 64.0,
                    )
                    if masked and s == 3:
                        # slot 3 (pipeline-fill phase): mask P on the DVE
                        nc.vector.scalar_tensor_tensor(
                            pt[:], r2_sb[:], b2t_sb[:, 14 + g:15 + g], pt[:],
                            op0=mybir.AluOpType.is_ge,
                            op1=mybir.AluOpType.mult,
                        )
                    pt_of[i] = pt

                def emit_AV(i):
                    # even k-tiles accumulate into otpA, odd into otpB, so
                    # consecutive matmuls hit different PSUM banks and their
                    # fill/drain phases overlap
                    s, g = groups[i]
                    pt = pt_of.pop(i)
                    for u in range(2):
                        kt = 2 * g + u
                        nc.tensor.matmul(
                            otp_of[s][u][:],
                            lhsT=vaug_sb[:, kt * 80:kt * 80 + 65],
                            rhs=pt[:, u * 512:(u + 1) * 512],
                            start=(kt == u),
                            stop=(kt == NKT[s] - 2 + u),
                        )

                epi_st = {}

                def emit_E_half(s, half):
                    last = s == SLOT_ORDER[-1]
                    if half == 0:
                        otpA, otpB = otp_of.pop(s)
                        ot16 = epi.tile([H + 1, 512], F16, name="ot16", tag="ot16")
                        dnrow = epi.tile([1, 512], F16, name="dnrow", tag="dnrow")
                        nc.vector.tensor_copy(ot16[:], otpA[:])
                        nc.vector.tensor_add(ot16[:], ot16[:], otpB[:])
                        nc.vector.tensor_copy(dnrow[:], ot16[64:65, :])
                        ysbw = ysbp.tile([128, 2048], F16, name="ysbw", tag="ysbw")
                        # all four denominator transposes into one PSUM tile,
                        # one batched reciprocal
                        dnp = projps.tile([128, 4], F32, name="pp", tag="pp")
                        for t in range(4):
                            nc.tensor.matmul(
                                dnp[:, t:t + 1],
                                lhsT=dnrow[:, t * 128:(t + 1) * 128],
                                rhs=ones_sb[:],
                                start=True, stop=True,
                            )
                        recip = epi.tile([128, 4], F32, name="recip", tag="recip")
                        nc.vector.reciprocal(recip[:], dnp[:])
                        epi_st[s] = (ot16, dnrow, ysbw, recip)
                    ot16, dnrow, ysbw, recip = epi_st[s]
                    for t in (0, 1) if half == 0 else (2, 3):
                        yp = projps.tile([128, 512], F32, name="pp", tag="pp")
                        nc.tensor.matmul(
                            yp[:],
                            lhsT=ot16[:, t * 128:(t + 1) * 128],
                            rhs=wpack_sb[0:65, 1024:1536],
                            start=True, stop=True,
                        )
                        if last and t in (1, 3):
                            # scalar engine is done with exps by now: use its
                            # free affine to scale, halving the tail chain
                            nc.scalar.activation(
                                ysbw[:, t * 512:(t + 1) * 512], yp[:],
                                mybir.ActivationFunctionType.Copy,
                                scale=recip[:, t:t + 1],
                            )
                        else:
                            nc.vector.tensor_scalar_mul(
                                ysbw[:, t * 512:(t + 1) * 512], yp[:],
                                recip[:, t:t + 1],
                            )
                        if last and t in (1, 3):
                            nc.sync.dma_start(
                                out=out_d[:, s * 2048 + (t - 1) * 512:
                                          s * 2048 + (t + 1) * 512],
                                in_=ysbw[:, (t - 1) * 512:(t + 1) * 512],
                            )
                    if half == 1 and not last:
                        nc.sync.dma_start(
                            out=out_d[:, s * 2048:(s + 1) * 2048],
                            in_=ysbw[:],
                        )

                # ---- software-pipelined emission ----
                prod = {
                    0: [lambda: emit_P(0), lambda: emit_Q(3)],
                    2: [lambda: emit_P(1), lambda: emit_Q(2),
                        lambda: emit_xtq_dma(1)],
                    4: [lambda: emit_P(2)],
                    8: [lambda: emit_P(3)],
                    12: [lambda: emit_Q(1), lambda: emit_P(4)],
                    16: [lambda: emit_P(5)],
                    20: [lambda: emit_P(6)],
                    24: [lambda: emit_Q(0)],
                    28: [lambda: emit_P(7)],
                }
                last_step_of_slot = {}
                acc = -1
                for s in SLOT_ORDER:
                    acc += NGRP[s]
                    last_step_of_slot[s] = acc

                n = len(groups)
                for i in range(n + 4):
                    for fn in prod.get(i, []):
                        fn()
                    if i < n:
                        emit_S(i)
                    if 0 <= i - 3 < n:
                        emit_AV(i - 3)
                        for s in SLOT_ORDER:
                            if last_step_of_slot[s] == i - 3:
                                emit_E_half(s, 0)
                    if 0 <= i - 4 < n:
                        for s in SLOT_ORDER:
                            if last_step_of_slot[s] == i - 4:
                                emit_E_half(s, 1)

    nc.compile()
    return nc


_NC_CACHE = {}


def _thr_row(blocks):
    # mask P[k_local, u*512+f] iff  f - p - 128*u < thr[s, j]
    # thr = 128*t0 - 512*block  with t0 = NKT[s]-8+2j  (even tile of group)
    t = np.zeros(16, np.float32)
    for s in range(4):
        for j in range(4):
            t0 = NKT[s] - 8 + 2 * j
            t[s * 4 + j] = 128.0 * t0 - 512.0 * blocks[s]
    return t


def _bias_tiles(blocks):
    # bias[p, idx*1024 + u*512+f] = NEG_BIAS where masked (slots 0-2 only)
    p = np.arange(128)[:, None, None]
    cols = np.arange(1024)[None, None, :]
    r2 = (cols % 512) - p - 128 * (cols // 512)
    thr = _thr_row(blocks)[:12].reshape(1, 12, 1)
    bias = np.where(r2 < thr, np.float32(NEG_BIAS), np.float32(0.0))
    return bias.reshape(128, 12 * 1024).astype(np.float16)


def _make_in_maps(x, Wq, bq, Wk, bk, Wv, bv, Wo, bo):
    # weight pack: [wkv(j-chunked) | wq2(j-chunked) | wo_aug padded]
    wkv = np.concatenate([Wv, Wk], axis=1).astype(np.float16)
    wkv = wkv.reshape(4, 128, 128).transpose(1, 0, 2).reshape(128, 512)
    wq2 = np.concatenate([Wq, Wq], axis=1).astype(np.float16)
    wq2 = wq2.reshape(4, 128, 128).transpose(1, 0, 2).reshape(128, 512)
    wo_aug = np.concatenate([Wo, (bv @ Wo + bo)[None, :]], axis=0)
    wop = np.zeros((128, 512), np.float16)
    wop[0:65] = wo_aug.astype(np.float16)
    wpack = np.ascontiguousarray(
        np.concatenate([wkv, wq2, wop], axis=1))           # (128, 1536)

    # biases + slot-3 thresholds: [bkv | bq2 | thr(16)]
    bkv = np.concatenate([np.zeros(64, np.float32), bk])
    bq2 = np.concatenate([bq, bq])
    thr_e, thr_o = _thr_row(BLOCKS_EVEN), _thr_row(BLOCKS_ODD)
    b2t_e = np.concatenate(
        [bkv[:, None], bq2[:, None], np.tile(thr_e[None, :], (128, 1))],
        axis=1).astype(np.float32)
    b2t_o = np.concatenate(
        [bkv[:, None], bq2[:, None], np.tile(thr_o[None, :], (128, 1))],
        axis=1).astype(np.float32)

    biasm_even = _bias_tiles(BLOCKS_EVEN)
    biasm_odd = _bias_tiles(BLOCKS_ODD)

    in_maps = []
    for c in range(8):
        b = c // 2
        blocks = BLOCKS_EVEN if c % 2 == 0 else BLOCKS_ODD
        xt = np.ascontiguousarray(x[b].T).astype(np.float16)      # (512, 4096)
        # xtb[p, sb*2048 + j*512 + cc] = xt[j*128+p, sb*512+cc]
        xtb = np.ascontiguousarray(
            xt.reshape(4, 128, 8, 512).transpose(1, 2, 0, 3).reshape(128, 16384))
        qcols = np.concatenate(
            [np.arange(blocks[s] * QB, (blocks[s] + 1) * QB) for s in SLOT_ORDER]
        )
        xtq = xt[:, qcols]                                        # (512, 2048)
        # xtqb[p, H*4096 + j*1024 + q*512 + cc] = xtq[j*128+p, (2H+q)*512+cc]
        xtqb = np.ascontiguousarray(
            xtq.reshape(4, 128, 2, 2, 512).transpose(1, 2, 0, 3, 4).reshape(128, 8192))
        in_maps.append({
            "xtb": xtb,
            "xtqb": xtqb,
            "wpack": wpack,
            "b2t": b2t_e if c % 2 == 0 else b2t_o,
            "biasm": biasm_even if c % 2 == 0 else biasm_odd,
        })
    return in_maps


def kernel(x, Wq, bq, Wk, bk, Wv, bv, Wo, bo):
    global LAST_EXEC_TIME_NS, LAST_RESULTS
    x = np.asarray(x, dtype=np.float32)
    Wq, bq = np.asarray(Wq, np.float32), np.asarray(bq, np.float32)
    Wk, bk = np.asarray(Wk, np.float32), np.asarray(bk, np.float32)
    Wv, bv = np.asarray(Wv, np.float32), np.asarray(bv, np.float32)
    Wo, bo = np.asarray(Wo, np.float32), np.asarray(bo, np.float32)

    if "nc" not in _NC_CACHE:
        _NC_CACHE["nc"] = _build_nc()
    nc = _NC_CACHE["nc"]

    in_maps = _make_in_maps(x, Wq, bq, Wk, bk, Wv, bv, Wo, bo)

    trace = os.environ.get("KERNEL_TRACE", "1") == "1"
    if trace:
        trace = _install_ntff_hook()
    tmpdir = os.environ.get("KERNEL_TRACE_DIR") or None
    try:
        res = run_bass_kernel_spmd(
            nc, in_maps, core_ids=list(range(8)), trace=trace, tmpdir=tmpdir
        )
    except Exception:
        if not trace:
            raise
        res = run_bass_kernel_spmd(nc, in_maps, core_ids=list(range(8)), trace=False)
    LAST_EXEC_TIME_NS = res.exec_time_ns
    LAST_RESULTS = res

    out = np.empty((B, S, D), np.float32)
    for c in range(8):
        b = c // 2
        blocks = BLOCKS_EVEN if c % 2 == 0 else BLOCKS_ODD
        shard2 = np.asarray(res.results[c]["out"], dtype=np.float32)
        # shard2[p, s*2048 + t*512 + cc] = y[slot s][t*128+p, cc]
        y = shard2.reshape(128, 4, 4, 512).transpose(1, 2, 0, 3).reshape(4, 512, 512)
        for s in range(4):
            out[b, blocks[s] * QB:(blocks[s] + 1) * QB, :] = y[s]
    return out


# revision 29
# speedup vs baseline: 1.0783x; 1.0024x over previous
"""Causal single-head attention layer on 8 TRN2 NeuronCores.

Reference (per batch b):
  Q = x@Wq+bq; K = x@Wk+bk; V = x@Wv+bv        (S=4096, D=512, H=64)
  S = Q K^T / sqrt(S);  P = softmax(S + causal_mask);  out = (P V) @ Wo + bo

Sharding: 8 cores = 4 batches x 2 "halves". Each core owns 4 query-blocks
of 512 rows of its batch: even cores take blocks [7,4,3,0], odd take
[6,5,2,1] (causal work 72 k-tiles each). SPMD requires one program, so
both core types run the same *structural* schedule with per-slot k-tile
counts NKT=[32,24,16,8]; over-structural/diagonal k-tiles are killed by
per-core mask data: slots 0-2 add shipped 0/-2048 bias tiles into the
scores on the PE (identity matmul), slot 3 (processed first, while the
PE pipeline is still filling) multiplies P by an on-chip ramp>=threshold
compare on the vector engine. No collectives are needed.

DMA strategy: a dma_start costs ~2us fixed + bytes/436GB/s and transfers
serialize per queue, and HWDGE (sync+scalar) completion semaphores
round-robin 8 shared lanes — so ship FEW, LARGE, host-packed 2D
transfers: one weight pack + x^T halves on the two HWDGE rings (8 HWDGE
DMAs total = no lane aliasing), xtq/bias-tiles/K^T-repacks on the gpsimd
SWDGE ring, and the output staged wide and shipped once per slot into a
host-unscrambled layout.

On-chip algorithm per core (all matmuls fp16, fp32 PSUM accumulate):
  xt (D-on-partition x^T, host-pretransposed) -> K^T,V^T proj (stacked
  [Wv|Wk] stationary) and Q^T proj on host-permuted xtq with duplicated
  [Wq|Wq] so Q^T lands on both partition halves.
  K^T is repacked (even k-tiles -> partitions 0:64, odd -> 64:128) so each
  S^T pair runs as two CONCURRENT PE row-tile matmuls (tile_position (0,0)
  and (64,0)), doubling S^T throughput.
  V^T -> V via PE transposes; V gets a ones column appended so the softmax
  denominator falls out of the AV matmul for free.
  Per group g: S^T [128k x 1024q] (+ masking) -> exp (ACT, scale 1/64) ->
  fp16 P -> AV accumulate out^T_aug [65, 512].
  Final: y = (out^T_aug.T @ [Wo; bv@Wo+bo]) * (1/denom).
  Softmax max-subtraction skipped: |S/64| <~ 1 so exp is safe.
  Slots are processed smallest-k-range first ([3,2,1,0]) so production
  stays ahead; emission is software-pipelined (AV lags S^T by 2 groups,
  projections interleaved, epilogues split in halves).
"""

import os
import math

os.environ.setdefault("MYCRO_LOCAL_CACHE", "1")

import numpy as np

import concourse.bass as bass
import concourse.mybir as mybir
import concourse.tile as tile
from concourse import bacc
from concourse.bass_utils import run_bass_kernel_spmd
from concourse.masks import make_identity

F32 = mybir.dt.float32
F16 = mybir.dt.float16
I16 = mybir.dt.int16

B, S, D, H = 4, 4096, 512, 64
QB = 512                  # query block
NKT = [32, 24, 16, 8]     # structural k-tiles (of 128) per slot
BLOCKS_EVEN = [7, 4, 3, 0]
BLOCKS_ODD = [6, 5, 2, 1]
NGRP = [n // 2 for n in NKT]          # groups (pairs of k-tiles) per slot
SLOT_ORDER = [3, 2, 1, 0]             # smallest k-range first
QPOS = {s: i for i, s in enumerate(SLOT_ORDER)}   # xtq column block of slot
NEG_BIAS = -2048.0                    # exp(-2048/64) == 0
N_DUMMY = 9                           # PE HAM warm-up matmuls

LAST_EXEC_TIME_NS = None
LAST_RESULTS = None


def _install_ntff_hook():
    """Register the axon NTFF profile hook if the image's antenv lacks it,
    so run_bass_kernel_spmd(trace=True) can report real exec_time_ns."""
    import sys
    import types
    try:
        from antenv.axon_hooks import get_axon_ntff_profile_hook  # noqa: F401
        return True  # already present
    except ImportError:
        pass
    try:
        import trn_agent_boot.trn_boot as _tb
        hook = _tb._ntff_profile_via_ctypes("/opt/axon/libaxon_pjrt.so")
        if hook is None:
            return False
        mod = types.ModuleType("antenv.axon_hooks")
        mod.get_axon_ntff_profile_hook = lambda: hook
        mod.set_axon_ntff_profile_hook = lambda h: None
        sys.modules["antenv.axon_hooks"] = mod
        return True
    except Exception:
        return False


def _build_nc():
    nc = bacc.Bacc(
        "TRN2",
        target_bir_lowering=False,
        debug=False,
        enable_asserts=False,
        num_devices=8,
    )

    # host-packed inputs (see _make_in_maps for the layouts)
    xtb_d = nc.dram_tensor("xtb", [128, 16384], F16, kind="ExternalInput")
    xtqb_d = nc.dram_tensor("xtqb", [128, 8192], F16, kind="ExternalInput")
    wpack_d = nc.dram_tensor("wpack", [128, 1536], F16, kind="ExternalInput")
    b2t_d = nc.dram_tensor("b2t", [128, 18], F32, kind="ExternalInput")
    biasm_d = nc.dram_tensor("biasm", [128, 12 * 1024], F16, kind="ExternalInput")
    out_d = nc.dram_tensor("out", [128, 8192], F16, kind="ExternalOutput")

    krepeat = int(os.environ.get("KREPEAT", "1"))
    with tile.TileContext(nc) as tc:
      for _rep in range(krepeat):
        with (
            tc.tile_pool(name="big", bufs=1) as big,
            tc.tile_pool(name="small", bufs=1) as small,
        ):
            # ---- persistent SBUF tensors ----
            xtb_sb = big.tile([128, 16384], F16, tag="xtb")
            xtqb_sb = big.tile([128, 8192], F16, tag="xtqb")
            kvt_sb = big.tile([128, S], F16, tag="kvt")     # 0:64 V^T, 64:128 K^T
            ktp_sb = big.tile([128, S // 2], F16, tag="ktp")  # packed K^T even|odd
            qtp_sb = big.tile([128, 4 * QB], F16, tag="qtp")  # Q^T dup halves
            vaug_sb = big.tile([128, 32 * 80], F16, tag="vaug")
            biasm_sb = big.tile([128, 12 * 1024], F16, tag="biasm")
            wpack_sb = small.tile([128, 1536], F16, tag="wpack")
            b2t_sb = small.tile([128, 18], F32, tag="b2t")
            ident_sb = small.tile([64, 64], F16, tag="ident")
            id128_sb = small.tile([128, 128], F16, tag="id128")
            ones_sb = small.tile([1, 1], F16, tag="ones")
            r2i_sb = small.tile([128, 1024], I16, tag="r2i")
            r2_sb = small.tile([128, 1024], F16, tag="r2")
            dummy_sb = small.tile([64, 512], F16, tag="dummy")
            warm_sb = small.tile([1, 2], F32, tag="warm")

            # ---- input DMAs: few, large, host-packed ----
            # xtb is block-major (col = sb*2048 + j*512 + c): the first two
            # 512KB transfers unblock KV blocks 0/1 as early as possible,
            # the rest rides in bigger chunks
            nc.scalar.dma_start(out=wpack_sb[:], in_=wpack_d[:, :])
            nc.scalar.dma_start(out=b2t_sb[:], in_=b2t_d[:, :])
            for idx0 in (8, 4, 0):
                nc.scalar.dma_start(
                    out=biasm_sb[:, idx0 * 1024:(idx0 + 4) * 1024],
                    in_=biasm_d[:, idx0 * 1024:(idx0 + 4) * 1024],
                )
            nc.scalar.dma_start(out=xtb_sb[:, 8192:12288],
                                in_=xtb_d[:, 8192:12288])
            nc.scalar.dma_start(out=xtb_sb[:, 12288:16384],
                                in_=xtb_d[:, 12288:16384])
            nc.sync.dma_start(out=xtb_sb[:, 0:4096], in_=xtb_d[:, 0:4096])
            nc.sync.dma_start(out=xtb_sb[:, 4096:8192], in_=xtb_d[:, 4096:8192])

            # ---- on-chip constants ----
            nc.vector.memset(dummy_sb[:], 0.0)
            nc.vector.memset(warm_sb[:, 0:1], 0.0)
            # preload the ACT exp table set before the real activations
            nc.scalar.activation(
                warm_sb[:, 1:2], warm_sb[:, 0:1],
                mybir.ActivationFunctionType.Exp,
            )
            make_identity(nc, ident_sb[:])
            make_identity(nc, id128_sb[:])
            nc.vector.memset(ones_sb[:], 1.0)
            # ramp R2[p, u*512+f] = f - p - 128*u  (slot-3 causal masking)
            nc.gpsimd.iota(
                r2i_sb[:], pattern=[[-128, 2], [1, 512]], base=0,
                channel_multiplier=-1,
            )
            nc.vector.tensor_copy(r2_sb[:], r2i_sb[:])
            vaug3 = vaug_sb[:].rearrange("p (k c) -> p k c", c=80)
            nc.vector.memset(vaug3[:, :, 64:65], 1.0)

            def emit_xtq_dma(h):
                # xtq is host-packed in slot processing order; half h=0
                # covers slots [3,2], h=1 covers [1,0]
                nc.gpsimd.dma_start(
                    out=xtqb_sb[:, h * 4096:(h + 1) * 4096],
                    in_=xtqb_d[:, h * 4096:(h + 1) * 4096],
                )

            emit_xtq_dma(0)

            with (
                tc.tile_pool(name="projps", bufs=2, space="PSUM") as projps,
                tc.tile_pool(name="stps", bufs=2, space="PSUM") as stps,
                tc.tile_pool(name="otps", bufs=1, space="PSUM") as otps,
                tc.tile_pool(name="ptp", bufs=8) as ptp,
                tc.tile_pool(name="epi", bufs=6) as epi,
                tc.tile_pool(name="ysbp", bufs=2) as ysbp,
            ):
                # PE HAM warm-up: dummy matmuls while input DMAs stream
                for _ in range(N_DUMMY):
                    dmy = projps.tile([64, 512], F32, name="pp", tag="pp")
                    nc.tensor.matmul(
                        dmy[:], lhsT=dummy_sb[:, 0:64], rhs=dummy_sb[:],
                        start=True, stop=True,
                    )

                kv_k = kvt_sb[64:128, :].rearrange(
                    "p (g u c) -> p g u c", u=2, c=128
                )

                def emit_KV(sb):
                    # KV projection for column block sb (k-tiles 4sb..4sb+3)
                    kvp = projps.tile([128, 512], F32, name="pp", tag="pp")
                    for j in range(4):
                        c0 = sb * 2048 + j * 512
                        nc.tensor.matmul(
                            kvp[:],
                            lhsT=wpack_sb[:, j * 128:(j + 1) * 128],
                            rhs=xtb_sb[:, c0:c0 + 512],
                            start=(j == 0),
                            stop=(j == 3),
                        )
                    nc.vector.tensor_scalar_add(
                        kvt_sb[:, sb * 512:(sb + 1) * 512], kvp[:],
                        b2t_sb[:, 0:1],
                    )

                def emit_VTPK(sb):
                    # repack K^T: even k-tiles -> partitions 0:64, odd -> 64:128
                    nc.gpsimd.dma_start(
                        out=ktp_sb[0:64, sb * 256:(sb + 1) * 256],
                        in_=kv_k[:, 2 * sb:2 * sb + 2, 0:1, :],
                    )
                    nc.gpsimd.dma_start(
                        out=ktp_sb[64:128, sb * 256:(sb + 1) * 256],
                        in_=kv_k[:, 2 * sb:2 * sb + 2, 1:2, :],
                    )
                    # V natural tiles via PE transpose
                    for kt in range(4 * sb, 4 * sb + 4):
                        vtp = projps.tile([128, 64], F16, name="pp", tag="pp")
                        nc.tensor.transpose(
                            vtp[:], kvt_sb[0:64, kt * 128:(kt + 1) * 128],
                            ident_sb[:],
                        )
                        nc.vector.tensor_copy(
                            vaug_sb[:, kt * 80:kt * 80 + 64], vtp[:]
                        )

                def emit_P(sb):
                    emit_KV(sb)
                    emit_VTPK(sb)

                def emit_Q(s):
                    hq, q2 = QPOS[s] // 2, QPOS[s] % 2
                    qp = projps.tile([128, 512], F32, name="pp", tag="pp")
                    for j in range(4):
                        c0 = hq * 4096 + j * 1024 + q2 * 512
                        nc.tensor.matmul(
                            qp[:],
                            lhsT=wpack_sb[:, 512 + j * 128:512 + (j + 1) * 128],
                            rhs=xtqb_sb[:, c0:c0 + 512],
                            start=(j == 0),
                            stop=(j == 3),
                        )
                    nc.vector.tensor_scalar_add(
                        qtp_sb[:, s * 512:(s + 1) * 512], qp[:],
                        b2t_sb[:, 1:2],
                    )

                groups = [(s, g) for s in SLOT_ORDER for g in range(NGRP[s])]
                otp_of = {}
                pt_of = {}

                def emit_S(i):
                    s, g = groups[i]
                    if g == 0:
                        otp_of[s] = (
                            otps.tile([H + 1, 512], F32, name="otpA", tag="otpA"),
                            otps.tile([H + 1, 512], F32, name="otpB", tag="otpB"),
                        )
                    masked = g >= NGRP[s] - 4
                    pe_mask = masked and s != 3
                    stp = stps.tile([128, 1024], F32, name="stp", tag="stp")
                    if s == 3:
                        # first slot: skip the K^T repack dependency, read
                        # K^T straight from kvt (both tiles on row group 64)
                        for u in range(2):
                            kt = 2 * g + u
                            nc.tensor.matmul(
                                stp[:, u * 512:(u + 1) * 512],
                                lhsT=kvt_sb[64:128, kt * 128:(kt + 1) * 128],
                                rhs=qtp_sb[64:128, s * 512:(s + 1) * 512],
                                start=True, stop=True,
                                tile_position=(64, 0),
                            )
                    else:
                        nc.tensor.matmul(
                            stp[:, 0:512],
                            lhsT=ktp_sb[0:64, g * 128:(g + 1) * 128],
                            rhs=qtp_sb[0:64, s * 512:(s + 1) * 512],
                            start=True, stop=not pe_mask,
                            tile_position=(0, 0),
                        )
                        nc.tensor.matmul(
                            stp[:, 512:1024],
                            lhsT=ktp_sb[64:128, g * 128:(g + 1) * 128],
                            rhs=qtp_sb[64:128, s * 512:(s + 1) * 512],
                            start=True, stop=not pe_mask,
                            tile_position=(64, 0),
                        )
                    if pe_mask:
                        # add 0/-2048 bias tiles into the scores on the PE
                        idx = s * 4 + (g - (NGRP[s] - 4))
                        for u in range(2):
                            nc.tensor.matmul(
                                stp[:, u * 512:(u + 1) * 512],
                                lhsT=id128_sb[:],
                                rhs=biasm_sb[:, idx * 1024 + u * 512:
                                             idx * 1024 + (u + 1) * 512],
                                start=False, stop=True,
                            )
                    pt = ptp.tile([128, 1024], F16, name="pt", tag="pt")
                    nc.scalar.activation(
                        pt[:], stp[:], mybir.ActivationFunctionType.Exp,
                        scale=1.0 / 64.0,
                    )
                    if masked and s == 3:
                        # slot 3 (pipeline-fill phase): mask P on the DVE
                        nc.vector.scalar_tensor_tensor(
                            pt[:], r2_sb[:], b2t_sb[:, 14 + g:15 + g], pt[:],
                            op0=mybir.AluOpType.is_ge,
                            op1=mybir.AluOpType.mult,
                        )
                    pt_of[i] = pt

                def emit_AV(i):
                    # even k-tiles accumulate into otpA, odd into otpB, so
                    # consecutive matmuls hit different PSUM banks and their
                    # fill/drain phases overlap
                    s, g = groups[i]
                    pt = pt_of.pop(i)
                    for u in range(2):
                        kt = 2 * g + u
                        nc.tensor.matmul(
                            otp_of[s][u][:],
                            lhsT=vaug_sb[:, kt * 80:kt * 80 + 65],
                            rhs=pt[:, u * 512:(u + 1) * 512],
                            start=(kt == u),
                            stop=(kt == NKT[s] - 2 + u),
                        )

                epi_st = {}

                def emit_E_half(s, half):
                    last = s == SLOT_ORDER[-1]
                    if half == 0:
                        otpA, otpB = otp_of.pop(s)
                        ot16 = epi.tile([H + 1, 512], F16, name="ot16", tag="ot16")
                        dnrow = epi.tile([1, 512], F16, name="dnrow", tag="dnrow")
                        nc.vector.tensor_copy(ot16[:], otpA[:])
                        nc.vector.tensor_add(ot16[:], ot16[:], otpB[:])
                        nc.vector.tensor_copy(dnrow[:], ot16[64:65, :])
                        ysbw = ysbp.tile([128, 2048], F16, name="ysbw", tag="ysbw")
                        # all four denominator transposes into one PSUM tile,
                        # one batched reciprocal
                        dnp = projps.tile([128, 4], F32, name="pp", tag="pp")
                        for t in range(4):
                            nc.tensor.matmul(
                                dnp[:, t:t + 1],
                                lhsT=dnrow[:, t * 128:(t + 1) * 128],
                                rhs=ones_sb[:],
                                start=True, stop=True,
                            )
                        recip = epi.tile([128, 4], F32, name="recip", tag="recip")
                        nc.vector.reciprocal(recip[:], dnp[:])
                        epi_st[s] = (ot16, dnrow, ysbw, recip)
                    ot16, dnrow, ysbw, recip = epi_st[s]
                    for t in (0, 1) if half == 0 else (2, 3):
                        yp = projps.tile([128, 512], F32, name="pp", tag="pp")
                        nc.tensor.matmul(
                            yp[:],
                            lhsT=ot16[:, t * 128:(t + 1) * 128],
                            rhs=wpack_sb[0:65, 1024:1536],
                            start=True, stop=True,
                        )
                        if last and t in (1, 3):
                            # scalar engine is done with exps by now: use its
                            # free affine to scale, halving the tail chain
                            nc.scalar.activation(
                                ysbw[:, t * 512:(t + 1) * 512], yp[:],
                                mybir.ActivationFunctionType.Copy,
                                scale=recip[:, t:t + 1],
                            )
                        else:
                            nc.vector.tensor_scalar_mul(
                                ysbw[:, t * 512:(t + 1) * 512], yp[:],
                                recip[:, t:t + 1],
                            )
                        if last and t in (1, 3):
                            nc.sync.dma_start(
                                out=out_d[:, s * 2048 + (t - 1) * 512:
                                          s * 2048 + (t + 1) * 512],
                                in_=ysbw[:, (t - 1) * 512:(t + 1) * 512],
                            )
                    if half == 1 and not last:
                        nc.sync.dma_start(
                            out=out_d[:, s * 2048:(s + 1) * 2048],
                            in_=ysbw[:],
                        )

                # ---- software-pipelined emission ----
                prod = {
                    0: [lambda: emit_P(0), lambda: emit_Q(3)],
                    2: [lambda: emit_P(1), lambda: emit_Q(2),
                        lambda: emit_xtq_dma(1)],
                    4: [lambda: emit_P(2)],
                    8: [lambda: emit_P(3)],
                    12: [lambda: emit_Q(1), lambda: emit_P(4)],
                    16: [lambda: emit_P(5)],
                    20: [lambda: emit_P(6)],
                    24: [lambda: emit_Q(0)],
                    28: [lambda: emit_P(7)],
                }
                last_step_of_slot = {}
                acc = -1
                for s in SLOT_ORDER:
                    acc += NGRP[s]
                    last_step_of_slot[s] = acc

                n = len(groups)
                for i in range(n + 5):
                    for fn in prod.get(i, []):
                        fn()
                    if i < n:
                        emit_S(i)
                    if 0 <= i - 4 < n:
                        emit_AV(i - 4)
                        for s in SLOT_ORDER:
                            if last_step_of_slot[s] == i - 4:
                                emit_E_half(s, 0)
                    if 0 <= i - 5 < n:
                        for s in SLOT_ORDER:
                            if last_step_of_slot[s] == i - 5:
                                emit_E_half(s, 1)

    nc.compile()
    return nc


_NC_CACHE = {}


def _thr_row(blocks):
    # mask P[k_local, u*512+f] iff  f - p - 128*u < thr[s, j]
    # thr = 128*t0 - 512*block  with t0 = NKT[s]-8+2j  (even tile of group)
    t = np.zeros(16, np.float32)
    for s in range(4):
        for j in range(4):
            t0 = NKT[s] - 8 + 2 * j
            t[s * 4 + j] = 128.0 * t0 - 512.0 * blocks[s]
    return t


def _bias_tiles(blocks):
    # bias[p, idx*1024 + u*512+f] = NEG_BIAS where masked (slots 0-2 only)
    p = np.arange(128)[:, None, None]
    cols = np.arange(1024)[None, None, :]
    r2 = (cols % 512) - p - 128 * (cols // 512)
    thr = _thr_row(blocks)[:12].reshape(1, 12, 1)
    bias = np.where(r2 < thr, np.float32(NEG_BIAS), np.float32(0.0))
    return bias.reshape(128, 12 * 1024).astype(np.float16)


def _make_in_maps(x, Wq, bq, Wk, bk, Wv, bv, Wo, bo):
    # weight pack: [wkv(j-chunked) | wq2(j-chunked) | wo_aug padded]
    wkv = np.concatenate([Wv, Wk], axis=1).astype(np.float16)
    wkv = wkv.reshape(4, 128, 128).transpose(1, 0, 2).reshape(128, 512)
    wq2 = np.concatenate([Wq, Wq], axis=1).astype(np.float16)
    wq2 = wq2.reshape(4, 128, 128).transpose(1, 0, 2).reshape(128, 512)
    wo_aug = np.concatenate([Wo, (bv @ Wo + bo)[None, :]], axis=0)
    wop = np.zeros((128, 512), np.float16)
    wop[0:65] = wo_aug.astype(np.float16)
    wpack = np.ascontiguousarray(
        np.concatenate([wkv, wq2, wop], axis=1))           # (128, 1536)

    # biases + slot-3 thresholds: [bkv | bq2 | thr(16)]
    bkv = np.concatenate([np.zeros(64, np.float32), bk])
    bq2 = np.concatenate([bq, bq])
    thr_e, thr_o = _thr_row(BLOCKS_EVEN), _thr_row(BLOCKS_ODD)
    b2t_e = np.concatenate(
        [bkv[:, None], bq2[:, None], np.tile(thr_e[None, :], (128, 1))],
        axis=1).astype(np.float32)
    b2t_o = np.concatenate(
        [bkv[:, None], bq2[:, None], np.tile(thr_o[None, :], (128, 1))],
        axis=1).astype(np.float32)

    biasm_even = _bias_tiles(BLOCKS_EVEN)
    biasm_odd = _bias_tiles(BLOCKS_ODD)

    in_maps = []
    for c in range(8):
        b = c // 2
        blocks = BLOCKS_EVEN if c % 2 == 0 else BLOCKS_ODD
        xt = np.ascontiguousarray(x[b].T).astype(np.float16)      # (512, 4096)
        # xtb[p, sb*2048 + j*512 + cc] = xt[j*128+p, sb*512+cc]
        xtb = np.ascontiguousarray(
            xt.reshape(4, 128, 8, 512).transpose(1, 2, 0, 3).reshape(128, 16384))
        qcols = np.concatenate(
            [np.arange(blocks[s] * QB, (blocks[s] + 1) * QB) for s in SLOT_ORDER]
        )
        xtq = xt[:, qcols]                                        # (512, 2048)
        # xtqb[p, H*4096 + j*1024 + q*512 + cc] = xtq[j*128+p, (2H+q)*512+cc]
        xtqb = np.ascontiguousarray(
            xtq.reshape(4, 128, 2, 2, 512).transpose(1, 2, 0, 3, 4).reshape(128, 8192))
        in_maps.append({
            "xtb": xtb,
            "xtqb": xtqb,
            "wpack": wpack,
            "b2t": b2t_e if c % 2 == 0 else b2t_o,
            "biasm": biasm_even if c % 2 == 0 else biasm_odd,
        })
    return in_maps


def kernel(x, Wq, bq, Wk, bk, Wv, bv, Wo, bo):
    global LAST_EXEC_TIME_NS, LAST_RESULTS
    x = np.asarray(x, dtype=np.float32)
    Wq, bq = np.asarray(Wq, np.float32), np.asarray(bq, np.float32)
    Wk, bk = np.asarray(Wk, np.float32), np.asarray(bk, np.float32)
    Wv, bv = np.asarray(Wv, np.float32), np.asarray(bv, np.float32)
    Wo, bo = np.asarray(Wo, np.float32), np.asarray(bo, np.float32)

    if "nc" not in _NC_CACHE:
        _NC_CACHE["nc"] = _build_nc()
    nc = _NC_CACHE["nc"]

    in_maps = _make_in_maps(x, Wq, bq, Wk, bk, Wv, bv, Wo, bo)

    trace = os.environ.get("KERNEL_TRACE", "1") == "1"
    if trace:
        trace = _install_ntff_hook()
    tmpdir = os.environ.get("KERNEL_TRACE_DIR") or None
    try:
        res = run_bass_kernel_spmd(
            nc, in_maps, core_ids=list(range(8)), trace=trace, tmpdir=tmpdir
        )
    except Exception:
        if not trace:
            raise
        res = run_bass_kernel_spmd(nc, in_maps, core_ids=list(range(8)), trace=False)
    LAST_EXEC_TIME_NS = res.exec_time_ns
    LAST_RESULTS = res

    out = np.empty((B, S, D), np.float32)
    for c in range(8):
        b = c // 2
        blocks = BLOCKS_EVEN if c % 2 == 0 else BLOCKS_ODD
        shard2 = np.asarray(res.results[c]["out"], dtype=np.float32)
        # shard2[p, s*2048 + t*512 + cc] = y[slot s][t*128+p, cc]
        y = shard2.reshape(128, 4, 4, 512).transpose(1, 2, 0, 3).reshape(4, 512, 512)
        for s in range(4):
            out[b, blocks[s] * QB:(blocks[s] + 1) * QB, :] = y[s]
    return out


# revision 30
# speedup vs baseline: 1.0921x; 1.0128x over previous
"""Causal single-head attention layer on 8 TRN2 NeuronCores.

Reference (per batch b):
  Q = x@Wq+bq; K = x@Wk+bk; V = x@Wv+bv        (S=4096, D=512, H=64)
  S = Q K^T / sqrt(S);  P = softmax(S + causal_mask);  out = (P V) @ Wo + bo

Sharding: 8 cores = 4 batches x 2 "halves". Each core owns 4 query-blocks
of 512 rows of its batch: even cores take blocks [7,4,3,0], odd take
[6,5,2,1] (causal work 72 k-tiles each). SPMD requires one program, so
both core types run the same *structural* schedule with per-slot k-tile
counts NKT=[32,24,16,8]; over-structural/diagonal k-tiles are killed by
per-core mask data: slots 0-2 add shipped 0/-2048 bias tiles into the
scores on the PE (identity matmul), slot 3 (processed first, while the
PE pipeline is still filling) multiplies P by an on-chip ramp>=threshold
compare on the vector engine. No collectives are needed.

DMA strategy: a dma_start costs ~2us fixed + bytes/436GB/s and transfers
serialize per queue, and HWDGE (sync+scalar) completion semaphores
round-robin 8 shared lanes — so ship FEW, LARGE, host-packed 2D
transfers: one weight pack + x^T halves on the two HWDGE rings (8 HWDGE
DMAs total = no lane aliasing), xtq/bias-tiles/K^T-repacks on the gpsimd
SWDGE ring, and the output staged wide and shipped once per slot into a
host-unscrambled layout.

On-chip algorithm per core (all matmuls fp16, fp32 PSUM accumulate):
  xt (D-on-partition x^T, host-pretransposed) -> K^T,V^T proj (stacked
  [Wv|Wk] stationary) and Q^T proj on host-permuted xtq with duplicated
  [Wq|Wq] so Q^T lands on both partition halves.
  K^T is repacked (even k-tiles -> partitions 0:64, odd -> 64:128) so each
  S^T pair runs as two CONCURRENT PE row-tile matmuls (tile_position (0,0)
  and (64,0)), doubling S^T throughput.
  V^T -> V via PE transposes; V gets a ones column appended so the softmax
  denominator falls out of the AV matmul for free.
  Per group g: S^T [128k x 1024q] (+ masking) -> exp (ACT, scale 1/64) ->
  fp16 P -> AV accumulate out^T_aug [65, 512].
  Final: y = (out^T_aug.T @ [Wo; bv@Wo+bo]) * (1/denom).
  Softmax max-subtraction skipped: |S/64| <~ 1 so exp is safe.
  Slots are processed smallest-k-range first ([3,2,1,0]) so production
  stays ahead; emission is software-pipelined (AV lags S^T by 2 groups,
  projections interleaved, epilogues split in halves).
"""

import os
import math

os.environ.setdefault("MYCRO_LOCAL_CACHE", "1")

import numpy as np

import concourse.bass as bass
import concourse.mybir as mybir
import concourse.tile as tile
from concourse import bacc
from concourse.bass_utils import run_bass_kernel_spmd
from concourse.masks import make_identity

F32 = mybir.dt.float32
F16 = mybir.dt.float16
I16 = mybir.dt.int16

B, S, D, H = 4, 4096, 512, 64
QB = 512                  # query block
NKT = [32, 24, 16, 8]     # structural k-tiles (of 128) per slot
BLOCKS_EVEN = [7, 4, 3, 0]
BLOCKS_ODD = [6, 5, 2, 1]
NGRP = [n // 2 for n in NKT]          # groups (pairs of k-tiles) per slot
SLOT_ORDER = [3, 2, 1, 0]             # smallest k-range first
QPOS = {s: i for i, s in enumerate(SLOT_ORDER)}   # xtq column block of slot
NEG_BIAS = -2048.0                    # exp(-2048/64) == 0
N_DUMMY = 9                           # PE HAM warm-up matmuls

LAST_EXEC_TIME_NS = None
LAST_RESULTS = None


def _install_ntff_hook():
    """Register the axon NTFF profile hook if the image's antenv lacks it,
    so run_bass_kernel_spmd(trace=True) can report real exec_time_ns."""
    import sys
    import types
    try:
        from antenv.axon_hooks import get_axon_ntff_profile_hook  # noqa: F401
        return True  # already present
    except ImportError:
        pass
    try:
        import trn_agent_boot.trn_boot as _tb
        hook = _tb._ntff_profile_via_ctypes("/opt/axon/libaxon_pjrt.so")
        if hook is None:
            return False
        mod = types.ModuleType("antenv.axon_hooks")
        mod.get_axon_ntff_profile_hook = lambda: hook
        mod.set_axon_ntff_profile_hook = lambda h: None
        sys.modules["antenv.axon_hooks"] = mod
        return True
    except Exception:
        return False


def _build_nc():
    nc = bacc.Bacc(
        "TRN2",
        target_bir_lowering=False,
        debug=False,
        enable_asserts=False,
        num_devices=8,
    )

    # host-packed inputs (see _make_in_maps for the layouts)
    xtb_d = nc.dram_tensor("xtb", [128, 16384], F16, kind="ExternalInput")
    xtqb_d = nc.dram_tensor("xtqb", [128, 8192], F16, kind="ExternalInput")
    wpack_d = nc.dram_tensor("wpack", [128, 1536], F16, kind="ExternalInput")
    b2t_d = nc.dram_tensor("b2t", [128, 18], F32, kind="ExternalInput")
    biasm_d = nc.dram_tensor("biasm", [128, 12 * 1024], F16, kind="ExternalInput")
    out_d = nc.dram_tensor("out", [128, 8192], F16, kind="ExternalOutput")

    krepeat = int(os.environ.get("KREPEAT", "1"))
    with tile.TileContext(nc) as tc:
      for _rep in range(krepeat):
        with (
            tc.tile_pool(name="big", bufs=1) as big,
            tc.tile_pool(name="small", bufs=1) as small,
        ):
            # ---- persistent SBUF tensors ----
            xtb_sb = big.tile([128, 16384], F16, tag="xtb")
            xtqb_sb = big.tile([128, 8192], F16, tag="xtqb")
            kvt_sb = big.tile([128, S], F16, tag="kvt")     # 0:64 V^T, 64:128 K^T
            ktp_sb = big.tile([128, S // 2], F16, tag="ktp")  # packed K^T even|odd
            qtp_sb = big.tile([128, 4 * QB], F16, tag="qtp")  # Q^T dup halves
            vaug_sb = big.tile([128, 32 * 80], F16, tag="vaug")
            biasm_sb = big.tile([128, 12 * 1024], F16, tag="biasm")
            wpack_sb = small.tile([128, 1536], F16, tag="wpack")
            b2t_sb = small.tile([128, 18], F32, tag="b2t")
            ident_sb = small.tile([64, 64], F16, tag="ident")
            id128_sb = small.tile([128, 128], F16, tag="id128")
            ones_sb = small.tile([1, 1], F16, tag="ones")
            r2i_sb = small.tile([128, 1024], I16, tag="r2i")
            r2_sb = small.tile([128, 1024], F16, tag="r2")
            dummy_sb = small.tile([64, 512], F16, tag="dummy")
            warm_sb = small.tile([1, 2], F32, tag="warm")

            # ---- input DMAs: few, large, host-packed ----
            # xtb is block-major (col = sb*2048 + j*512 + c): the first two
            # 512KB transfers unblock KV blocks 0/1 as early as possible,
            # the rest rides in bigger chunks
            nc.scalar.dma_start(out=wpack_sb[:], in_=wpack_d[:, :])
            nc.scalar.dma_start(out=b2t_sb[:], in_=b2t_d[:, :])
            for idx0 in (4, 0):
                nc.scalar.dma_start(
                    out=biasm_sb[:, idx0 * 1024:(idx0 + 4) * 1024],
                    in_=biasm_d[:, idx0 * 1024:(idx0 + 4) * 1024],
                )
            nc.scalar.dma_start(out=xtb_sb[:, 8192:12288],
                                in_=xtb_d[:, 8192:12288])
            nc.scalar.dma_start(out=xtb_sb[:, 12288:16384],
                                in_=xtb_d[:, 12288:16384])
            nc.sync.dma_start(out=xtb_sb[:, 0:4096], in_=xtb_d[:, 0:4096])
            nc.sync.dma_start(out=xtb_sb[:, 4096:8192], in_=xtb_d[:, 4096:8192])

            # ---- on-chip constants ----
            nc.vector.memset(dummy_sb[:], 0.0)
            nc.vector.memset(warm_sb[:, 0:1], 0.0)
            # preload the ACT exp table set before the real activations
            nc.scalar.activation(
                warm_sb[:, 1:2], warm_sb[:, 0:1],
                mybir.ActivationFunctionType.Exp,
            )
            make_identity(nc, ident_sb[:])
            make_identity(nc, id128_sb[:])
            nc.vector.memset(ones_sb[:], 1.0)
            # ramp R2[p, u*512+f] = f - p - 128*u  (slot-3 causal masking)
            nc.gpsimd.iota(
                r2i_sb[:], pattern=[[-128, 2], [1, 512]], base=0,
                channel_multiplier=-1,
            )
            nc.vector.tensor_copy(r2_sb[:], r2i_sb[:])
            vaug3 = vaug_sb[:].rearrange("p (k c) -> p k c", c=80)
            nc.vector.memset(vaug3[:, :, 64:65], 1.0)

            def emit_xtq_dma(h):
                # xtq is host-packed in slot processing order; half h=0
                # covers slots [3,2], h=1 covers [1,0]
                nc.gpsimd.dma_start(
                    out=xtqb_sb[:, h * 4096:(h + 1) * 4096],
                    in_=xtqb_d[:, h * 4096:(h + 1) * 4096],
                )

            emit_xtq_dma(0)

            with (
                tc.tile_pool(name="projps", bufs=2, space="PSUM") as projps,
                tc.tile_pool(name="stps", bufs=2, space="PSUM") as stps,
                tc.tile_pool(name="otps", bufs=1, space="PSUM") as otps,
                tc.tile_pool(name="ptp", bufs=8) as ptp,
                tc.tile_pool(name="epi", bufs=6) as epi,
                tc.tile_pool(name="ysbp", bufs=2) as ysbp,
            ):
                # PE HAM warm-up: dummy matmuls while input DMAs stream
                for _ in range(N_DUMMY):
                    dmy = projps.tile([64, 512], F32, name="pp", tag="pp")
                    nc.tensor.matmul(
                        dmy[:], lhsT=dummy_sb[:, 0:64], rhs=dummy_sb[:],
                        start=True, stop=True,
                    )

                kv_k = kvt_sb[64:128, :].rearrange(
                    "p (g u c) -> p g u c", u=2, c=128
                )

                def emit_KV(sb):
                    # KV projection for column block sb (k-tiles 4sb..4sb+3)
                    kvp = projps.tile([128, 512], F32, name="pp", tag="pp")
                    for j in range(4):
                        c0 = sb * 2048 + j * 512
                        nc.tensor.matmul(
                            kvp[:],
                            lhsT=wpack_sb[:, j * 128:(j + 1) * 128],
                            rhs=xtb_sb[:, c0:c0 + 512],
                            start=(j == 0),
                            stop=(j == 3),
                        )
                    nc.vector.tensor_scalar_add(
                        kvt_sb[:, sb * 512:(sb + 1) * 512], kvp[:],
                        b2t_sb[:, 0:1],
                    )

                def emit_VTPK(sb):
                    # repack K^T: even k-tiles -> partitions 0:64, odd -> 64:128
                    nc.gpsimd.dma_start(
                        out=ktp_sb[0:64, sb * 256:(sb + 1) * 256],
                        in_=kv_k[:, 2 * sb:2 * sb + 2, 0:1, :],
                    )
                    nc.gpsimd.dma_start(
                        out=ktp_sb[64:128, sb * 256:(sb + 1) * 256],
                        in_=kv_k[:, 2 * sb:2 * sb + 2, 1:2, :],
                    )
                    # V natural tiles via PE transpose
                    for kt in range(4 * sb, 4 * sb + 4):
                        vtp = projps.tile([128, 64], F16, name="pp", tag="pp")
                        nc.tensor.transpose(
                            vtp[:], kvt_sb[0:64, kt * 128:(kt + 1) * 128],
                            ident_sb[:],
                        )
                        nc.vector.tensor_copy(
                            vaug_sb[:, kt * 80:kt * 80 + 64], vtp[:]
                        )

                def emit_P(sb):
                    emit_KV(sb)
                    emit_VTPK(sb)

                def emit_Q(s):
                    hq, q2 = QPOS[s] // 2, QPOS[s] % 2
                    qp = projps.tile([128, 512], F32, name="pp", tag="pp")
                    for j in range(4):
                        c0 = hq * 4096 + j * 1024 + q2 * 512
                        nc.tensor.matmul(
                            qp[:],
                            lhsT=wpack_sb[:, 512 + j * 128:512 + (j + 1) * 128],
                            rhs=xtqb_sb[:, c0:c0 + 512],
                            start=(j == 0),
                            stop=(j == 3),
                        )
                    nc.vector.tensor_scalar_add(
                        qtp_sb[:, s * 512:(s + 1) * 512], qp[:],
                        b2t_sb[:, 1:2],
                    )

                groups = [(s, g) for s in SLOT_ORDER for g in range(NGRP[s])]
                otp_of = {}
                pt_of = {}

                def emit_S(i):
                    s, g = groups[i]
                    if g == 0:
                        otp_of[s] = (
                            otps.tile([H + 1, 512], F32, name="otpA", tag="otpA"),
                            otps.tile([H + 1, 512], F32, name="otpB", tag="otpB"),
                        )
                    masked = g >= NGRP[s] - 4
                    pe_mask = masked and s not in (2, 3)
                    stp = stps.tile([128, 1024], F32, name="stp", tag="stp")
                    if s == 3:
                        # first slot: skip the K^T repack dependency, read
                        # K^T straight from kvt (both tiles on row group 64)
                        for u in range(2):
                            kt = 2 * g + u
                            nc.tensor.matmul(
                                stp[:, u * 512:(u + 1) * 512],
                                lhsT=kvt_sb[64:128, kt * 128:(kt + 1) * 128],
                                rhs=qtp_sb[64:128, s * 512:(s + 1) * 512],
                                start=True, stop=True,
                                tile_position=(64, 0),
                            )
                    else:
                        nc.tensor.matmul(
                            stp[:, 0:512],
                            lhsT=ktp_sb[0:64, g * 128:(g + 1) * 128],
                            rhs=qtp_sb[0:64, s * 512:(s + 1) * 512],
                            start=True, stop=not pe_mask,
                            tile_position=(0, 0),
                        )
                        nc.tensor.matmul(
                            stp[:, 512:1024],
                            lhsT=ktp_sb[64:128, g * 128:(g + 1) * 128],
                            rhs=qtp_sb[64:128, s * 512:(s + 1) * 512],
                            start=True, stop=not pe_mask,
                            tile_position=(64, 0),
                        )
                    if pe_mask:
                        # add 0/-2048 bias tiles into the scores on the PE
                        idx = s * 4 + (g - (NGRP[s] - 4))
                        for u in range(2):
                            nc.tensor.matmul(
                                stp[:, u * 512:(u + 1) * 512],
                                lhsT=id128_sb[:],
                                rhs=biasm_sb[:, idx * 1024 + u * 512:
                                             idx * 1024 + (u + 1) * 512],
                                start=False, stop=True,
                            )
                    pt = ptp.tile([128, 1024], F16, name="pt", tag="pt")
                    nc.scalar.activation(
                        pt[:], stp[:], mybir.ActivationFunctionType.Exp,
                        scale=1.0 / 64.0,
                    )
                    if masked and not pe_mask:
                        # slots 3/2: mask P on the DVE (idle there); the AV
                        # lag covers the chain latency
                        idx = s * 4 + (g - (NGRP[s] - 4))
                        nc.vector.scalar_tensor_tensor(
                            pt[:], r2_sb[:], b2t_sb[:, 2 + idx:3 + idx], pt[:],
                            op0=mybir.AluOpType.is_ge,
                            op1=mybir.AluOpType.mult,
                        )
                    pt_of[i] = pt

                def emit_AV(i):
                    # even k-tiles accumulate into otpA, odd into otpB, so
                    # consecutive matmuls hit different PSUM banks and their
                    # fill/drain phases overlap
                    s, g = groups[i]
                    pt = pt_of.pop(i)
                    for u in range(2):
                        kt = 2 * g + u
                        nc.tensor.matmul(
                            otp_of[s][u][:],
                            lhsT=vaug_sb[:, kt * 80:kt * 80 + 65],
                            rhs=pt[:, u * 512:(u + 1) * 512],
                            start=(kt == u),
                            stop=(kt == NKT[s] - 2 + u),
                        )

                epi_st = {}

                def emit_E_half(s, half):
                    last = s == SLOT_ORDER[-1]
                    if half == 0:
                        otpA, otpB = otp_of.pop(s)
                        ot16 = epi.tile([H + 1, 512], F16, name="ot16", tag="ot16")
                        dnrow = epi.tile([1, 512], F16, name="dnrow", tag="dnrow")
                        nc.vector.tensor_copy(ot16[:], otpA[:])
                        nc.vector.tensor_add(ot16[:], ot16[:], otpB[:])
                        nc.vector.tensor_copy(dnrow[:], ot16[64:65, :])
                        ysbw = ysbp.tile([128, 2048], F16, name="ysbw", tag="ysbw")
                        # all four denominator transposes into one PSUM tile,
                        # one batched reciprocal
                        dnp = projps.tile([128, 4], F32, name="pp", tag="pp")
                        for t in range(4):
                            nc.tensor.matmul(
                                dnp[:, t:t + 1],
                                lhsT=dnrow[:, t * 128:(t + 1) * 128],
                                rhs=ones_sb[:],
                                start=True, stop=True,
                            )
                        recip = epi.tile([128, 4], F32, name="recip", tag="recip")
                        nc.vector.reciprocal(recip[:], dnp[:])
                        epi_st[s] = (ot16, dnrow, ysbw, recip)
                    ot16, dnrow, ysbw, recip = epi_st[s]
                    for t in (0, 1) if half == 0 else (2, 3):
                        yp = projps.tile([128, 512], F32, name="pp", tag="pp")
                        nc.tensor.matmul(
                            yp[:],
                            lhsT=ot16[:, t * 128:(t + 1) * 128],
                            rhs=wpack_sb[0:65, 1024:1536],
                            start=True, stop=True,
                        )
                        if last and t in (1, 3):
                            # scalar engine is done with exps by now: use its
                            # free affine to scale, halving the tail chain
                            nc.scalar.activation(
                                ysbw[:, t * 512:(t + 1) * 512], yp[:],
                                mybir.ActivationFunctionType.Copy,
                                scale=recip[:, t:t + 1],
                            )
                        else:
                            nc.vector.tensor_scalar_mul(
                                ysbw[:, t * 512:(t + 1) * 512], yp[:],
                                recip[:, t:t + 1],
                            )
                        if last and t in (1, 3):
                            nc.sync.dma_start(
                                out=out_d[:, s * 2048 + (t - 1) * 512:
                                          s * 2048 + (t + 1) * 512],
                                in_=ysbw[:, (t - 1) * 512:(t + 1) * 512],
                            )
                    if half == 1 and not last:
                        nc.sync.dma_start(
                            out=out_d[:, s * 2048:(s + 1) * 2048],
                            in_=ysbw[:],
                        )

                # ---- software-pipelined emission ----
                prod = {
                    0: [lambda: emit_P(0), lambda: emit_Q(3)],
                    2: [lambda: emit_P(1), lambda: emit_Q(2),
                        lambda: emit_xtq_dma(1)],
                    4: [lambda: emit_P(2)],
                    8: [lambda: emit_P(3)],
                    12: [lambda: emit_Q(1), lambda: emit_P(4)],
                    16: [lambda: emit_P(5)],
                    20: [lambda: emit_P(6)],
                    24: [lambda: emit_Q(0)],
                    28: [lambda: emit_P(7)],
                }
                last_step_of_slot = {}
                acc = -1
                for s in SLOT_ORDER:
                    acc += NGRP[s]
                    last_step_of_slot[s] = acc

                n = len(groups)
                for i in range(n + 5):
                    for fn in prod.get(i, []):
                        fn()
                    if i < n:
                        emit_S(i)
                    if 0 <= i - 4 < n:
                        emit_AV(i - 4)
                        for s in SLOT_ORDER:
                            if last_step_of_slot[s] == i - 4:
                                emit_E_half(s, 0)
                    if 0 <= i - 5 < n:
                        for s in SLOT_ORDER:
                            if last_step_of_slot[s] == i - 5:
                                emit_E_half(s, 1)

    nc.compile()
    return nc


_NC_CACHE = {}


def _thr_row(blocks):
    # mask P[k_local, u*512+f] iff  f - p - 128*u < thr[s, j]
    # thr = 128*t0 - 512*block  with t0 = NKT[s]-8+2j  (even tile of group)
    t = np.zeros(16, np.float32)
    for s in range(4):
        for j in range(4):
            t0 = NKT[s] - 8 + 2 * j
            t[s * 4 + j] = 128.0 * t0 - 512.0 * blocks[s]
    return t


def _bias_tiles(blocks):
    # bias[p, idx*1024 + u*512+f] = NEG_BIAS where masked (slots 0-2 only)
    p = np.arange(128)[:, None, None]
    cols = np.arange(1024)[None, None, :]
    r2 = (cols % 512) - p - 128 * (cols // 512)
    thr = _thr_row(blocks)[:12].reshape(1, 12, 1)
    bias = np.where(r2 < thr, np.float32(NEG_BIAS), np.float32(0.0))
    return bias.reshape(128, 12 * 1024).astype(np.float16)


def _make_in_maps(x, Wq, bq, Wk, bk, Wv, bv, Wo, bo):
    # weight pack: [wkv(j-chunked) | wq2(j-chunked) | wo_aug padded]
    wkv = np.concatenate([Wv, Wk], axis=1).astype(np.float16)
    wkv = wkv.reshape(4, 128, 128).transpose(1, 0, 2).reshape(128, 512)
    wq2 = np.concatenate([Wq, Wq], axis=1).astype(np.float16)
    wq2 = wq2.reshape(4, 128, 128).transpose(1, 0, 2).reshape(128, 512)
    wo_aug = np.concatenate([Wo, (bv @ Wo + bo)[None, :]], axis=0)
    wop = np.zeros((128, 512), np.float16)
    wop[0:65] = wo_aug.astype(np.float16)
    wpack = np.ascontiguousarray(
        np.concatenate([wkv, wq2, wop], axis=1))           # (128, 1536)

    # biases + slot-3 thresholds: [bkv | bq2 | thr(16)]
    bkv = np.concatenate([np.zeros(64, np.float32), bk])
    bq2 = np.concatenate([bq, bq])
    thr_e, thr_o = _thr_row(BLOCKS_EVEN), _thr_row(BLOCKS_ODD)
    b2t_e = np.concatenate(
        [bkv[:, None], bq2[:, None], np.tile(thr_e[None, :], (128, 1))],
        axis=1).astype(np.float32)
    b2t_o = np.concatenate(
        [bkv[:, None], bq2[:, None], np.tile(thr_o[None, :], (128, 1))],
        axis=1).astype(np.float32)

    biasm_even = _bias_tiles(BLOCKS_EVEN)
    biasm_odd = _bias_tiles(BLOCKS_ODD)

    in_maps = []
    for c in range(8):
        b = c // 2
        blocks = BLOCKS_EVEN if c % 2 == 0 else BLOCKS_ODD
        xt = np.ascontiguousarray(x[b].T).astype(np.float16)      # (512, 4096)
        # xtb[p, sb*2048 + j*512 + cc] = xt[j*128+p, sb*512+cc]
        xtb = np.ascontiguousarray(
            xt.reshape(4, 128, 8, 512).transpose(1, 2, 0, 3).reshape(128, 16384))
        qcols = np.concatenate(
            [np.arange(blocks[s] * QB, (blocks[s] + 1) * QB) for s in SLOT_ORDER]
        )
        xtq = xt[:, qcols]                                        # (512, 2048)
        # xtqb[p, H*4096 + j*1024 + q*512 + cc] = xtq[j*128+p, (2H+q)*512+cc]
        xtqb = np.ascontiguousarray(
            xtq.reshape(4, 128, 2, 2, 512).transpose(1, 2, 0, 3, 4).reshape(128, 8192))
        in_maps.append({
            "xtb": xtb,
            "xtqb": xtqb,
            "wpack": wpack,
            "b2t": b2t_e if c % 2 == 0 else b2t_o,
            "biasm": biasm_even if c % 2 == 0 else biasm_odd,
        })
    return in_maps


def kernel(x, Wq, bq, Wk, bk, Wv, bv, Wo, bo):
    global LAST_EXEC_TIME_NS, LAST_RESULTS
    x = np.asarray(x, dtype=np.float32)
    Wq, bq = np.asarray(Wq, np.float32), np.asarray(bq, np.float32)
    Wk, bk = np.asarray(Wk, np.float32), np.asarray(bk, np.float32)
    Wv, bv = np.asarray(Wv, np.float32), np.asarray(bv, np.float32)
    Wo, bo = np.asarray(Wo, np.float32), np.asarray(bo, np.float32)

    if "nc" not in _NC_CACHE:
        _NC_CACHE["nc"] = _build_nc()
    nc = _NC_CACHE["nc"]

    in_maps = _make_in_maps(x, Wq, bq, Wk, bk, Wv, bv, Wo, bo)

    trace = os.environ.get("KERNEL_TRACE", "1") == "1"
    if trace:
        trace = _install_ntff_hook()
    tmpdir = os.environ.get("KERNEL_TRACE_DIR") or None
    try:
        res = run_bass_kernel_spmd(
            nc, in_maps, core_ids=list(range(8)), trace=trace, tmpdir=tmpdir
        )
    except Exception:
        if not trace:
            raise
        res = run_bass_kernel_spmd(nc, in_maps, core_ids=list(range(8)), trace=False)
    LAST_EXEC_TIME_NS = res.exec_time_ns
    LAST_RESULTS = res

    out = np.empty((B, S, D), np.float32)
    for c in range(8):
        b = c // 2
        blocks = BLOCKS_EVEN if c % 2 == 0 else BLOCKS_ODD
        shard2 = np.asarray(res.results[c]["out"], dtype=np.float32)
        # shard2[p, s*2048 + t*512 + cc] = y[slot s][t*128+p, cc]
        y = shard2.reshape(128, 4, 4, 512).transpose(1, 2, 0, 3).reshape(4, 512, 512)
        for s in range(4):
            out[b, blocks[s] * QB:(blocks[s] + 1) * QB, :] = y[s]
    return out


# revision 31
# speedup vs baseline: 1.0947x; 1.0024x over previous
"""Causal single-head attention layer on 8 TRN2 NeuronCores.

Reference (per batch b):
  Q = x@Wq+bq; K = x@Wk+bk; V = x@Wv+bv        (S=4096, D=512, H=64)
  S = Q K^T / sqrt(S);  P = softmax(S + causal_mask);  out = (P V) @ Wo + bo

Sharding: 8 cores = 4 batches x 2 "halves". Each core owns 4 query-blocks
of 512 rows of its batch: even cores take blocks [7,4,3,0], odd take
[6,5,2,1] (causal work 72 k-tiles each). SPMD requires one program, so
both core types run the same *structural* schedule with per-slot k-tile
counts NKT=[32,24,16,8]; over-structural/diagonal k-tiles are killed by
per-core mask data: slots 0-2 add shipped 0/-2048 bias tiles into the
scores on the PE (identity matmul), slot 3 (processed first, while the
PE pipeline is still filling) multiplies P by an on-chip ramp>=threshold
compare on the vector engine. No collectives are needed.

DMA strategy: a dma_start costs ~2us fixed + bytes/436GB/s and transfers
serialize per queue, and HWDGE (sync+scalar) completion semaphores
round-robin 8 shared lanes — so ship FEW, LARGE, host-packed 2D
transfers: one weight pack + x^T halves on the two HWDGE rings (8 HWDGE
DMAs total = no lane aliasing), xtq/bias-tiles/K^T-repacks on the gpsimd
SWDGE ring, and the output staged wide and shipped once per slot into a
host-unscrambled layout.

On-chip algorithm per core (all matmuls fp16, fp32 PSUM accumulate):
  xt (D-on-partition x^T, host-pretransposed) -> K^T,V^T proj (stacked
  [Wv|Wk] stationary) and Q^T proj on host-permuted xtq with duplicated
  [Wq|Wq] so Q^T lands on both partition halves.
  K^T is repacked (even k-tiles -> partitions 0:64, odd -> 64:128) so each
  S^T pair runs as two CONCURRENT PE row-tile matmuls (tile_position (0,0)
  and (64,0)), doubling S^T throughput.
  V^T -> V via PE transposes; V gets a ones column appended so the softmax
  denominator falls out of the AV matmul for free.
  Per group g: S^T [128k x 1024q] (+ masking) -> exp (ACT, scale 1/64) ->
  fp16 P -> AV accumulate out^T_aug [65, 512].
  Final: y = (out^T_aug.T @ [Wo; bv@Wo+bo]) * (1/denom).
  Softmax max-subtraction skipped: |S/64| <~ 1 so exp is safe.
  Slots are processed smallest-k-range first ([3,2,1,0]) so production
  stays ahead; emission is software-pipelined (AV lags S^T by 2 groups,
  projections interleaved, epilogues split in halves).
"""

import os
import math

os.environ.setdefault("MYCRO_LOCAL_CACHE", "1")

import numpy as np

import concourse.bass as bass
import concourse.mybir as mybir
import concourse.tile as tile
from concourse import bacc
from concourse.bass_utils import run_bass_kernel_spmd
from concourse.masks import make_identity

F32 = mybir.dt.float32
F16 = mybir.dt.float16
I16 = mybir.dt.int16

B, S, D, H = 4, 4096, 512, 64
QB = 512                  # query block
NKT = [32, 24, 16, 8]     # structural k-tiles (of 128) per slot
BLOCKS_EVEN = [7, 4, 3, 0]
BLOCKS_ODD = [6, 5, 2, 1]
NGRP = [n // 2 for n in NKT]          # groups (pairs of k-tiles) per slot
SLOT_ORDER = [3, 2, 1, 0]             # smallest k-range first
QPOS = {s: i for i, s in enumerate(SLOT_ORDER)}   # xtq column block of slot
NEG_BIAS = -2048.0                    # exp(-2048/64) == 0
N_DUMMY = 9                           # PE HAM warm-up matmuls

LAST_EXEC_TIME_NS = None
LAST_RESULTS = None


def _install_ntff_hook():
    """Register the axon NTFF profile hook if the image's antenv lacks it,
    so run_bass_kernel_spmd(trace=True) can report real exec_time_ns."""
    import sys
    import types
    try:
        from antenv.axon_hooks import get_axon_ntff_profile_hook  # noqa: F401
        return True  # already present
    except ImportError:
        pass
    try:
        import trn_agent_boot.trn_boot as _tb
        hook = _tb._ntff_profile_via_ctypes("/opt/axon/libaxon_pjrt.so")
        if hook is None:
            return False
        mod = types.ModuleType("antenv.axon_hooks")
        mod.get_axon_ntff_profile_hook = lambda: hook
        mod.set_axon_ntff_profile_hook = lambda h: None
        sys.modules["antenv.axon_hooks"] = mod
        return True
    except Exception:
        return False


def _build_nc():
    nc = bacc.Bacc(
        "TRN2",
        target_bir_lowering=False,
        debug=False,
        enable_asserts=False,
        num_devices=8,
    )

    # host-packed inputs (see _make_in_maps for the layouts)
    xtb_d = nc.dram_tensor("xtb", [128, 16384], F16, kind="ExternalInput")
    xtqb_d = nc.dram_tensor("xtqb", [128, 8192], F16, kind="ExternalInput")
    wpack_d = nc.dram_tensor("wpack", [128, 1536], F16, kind="ExternalInput")
    b2t_d = nc.dram_tensor("b2t", [128, 18], F32, kind="ExternalInput")
    biasm_d = nc.dram_tensor("biasm", [128, 12 * 1024], F16, kind="ExternalInput")
    out_d = nc.dram_tensor("out", [128, 8192], F16, kind="ExternalOutput")

    krepeat = int(os.environ.get("KREPEAT", "1"))
    with tile.TileContext(nc) as tc:
      for _rep in range(krepeat):
        with (
            tc.tile_pool(name="big", bufs=1) as big,
            tc.tile_pool(name="small", bufs=1) as small,
        ):
            # ---- persistent SBUF tensors ----
            xtb_sb = big.tile([128, 16384], F16, tag="xtb")
            xtqb_sb = big.tile([128, 8192], F16, tag="xtqb")
            kvt_sb = big.tile([128, S], F16, tag="kvt")     # 0:64 V^T, 64:128 K^T
            ktp_sb = big.tile([128, S // 2], F16, tag="ktp")  # packed K^T even|odd
            qtp_sb = big.tile([128, 4 * QB], F16, tag="qtp")  # Q^T dup halves
            vaug_sb = big.tile([128, 32 * 80], F16, tag="vaug")
            biasm_sb = big.tile([128, 12 * 1024], F16, tag="biasm")
            wpack_sb = small.tile([128, 1536], F16, tag="wpack")
            b2t_sb = small.tile([128, 18], F32, tag="b2t")
            ident_sb = small.tile([64, 64], F16, tag="ident")
            id128_sb = small.tile([128, 128], F16, tag="id128")
            ones_sb = small.tile([1, 1], F16, tag="ones")
            r2i_sb = small.tile([128, 1024], I16, tag="r2i")
            r2_sb = small.tile([128, 1024], F16, tag="r2")
            dummy_sb = small.tile([128, 512], F16, tag="dummy")
            warm_sb = small.tile([1, 2], F32, tag="warm")

            # ---- input DMAs: few, large, host-packed ----
            # xtb is block-major (col = sb*2048 + j*512 + c): the first two
            # 512KB transfers unblock KV blocks 0/1 as early as possible,
            # the rest rides in bigger chunks
            nc.scalar.dma_start(out=wpack_sb[:], in_=wpack_d[:, :])
            nc.scalar.dma_start(out=b2t_sb[:], in_=b2t_d[:, :])
            for idx0 in (8, 4, 0):
                nc.scalar.dma_start(
                    out=biasm_sb[:, idx0 * 1024:(idx0 + 4) * 1024],
                    in_=biasm_d[:, idx0 * 1024:(idx0 + 4) * 1024],
                )
            nc.scalar.dma_start(out=xtb_sb[:, 8192:12288],
                                in_=xtb_d[:, 8192:12288])
            nc.scalar.dma_start(out=xtb_sb[:, 12288:16384],
                                in_=xtb_d[:, 12288:16384])
            nc.sync.dma_start(out=xtb_sb[:, 0:4096], in_=xtb_d[:, 0:4096])
            nc.sync.dma_start(out=xtb_sb[:, 4096:8192], in_=xtb_d[:, 4096:8192])

            # ---- on-chip constants ----
            nc.vector.memset(dummy_sb[:], 0.0)
            nc.vector.memset(warm_sb[:, 0:1], 0.0)
            # preload the ACT exp table set before the real activations
            nc.scalar.activation(
                warm_sb[:, 1:2], warm_sb[:, 0:1],
                mybir.ActivationFunctionType.Exp,
            )
            make_identity(nc, ident_sb[:])
            make_identity(nc, id128_sb[:])
            nc.vector.memset(ones_sb[:], 1.0)
            # ramp R2[p, u*512+f] = f - p - 128*u  (slot-3 causal masking)
            nc.gpsimd.iota(
                r2i_sb[:], pattern=[[-128, 2], [1, 512]], base=0,
                channel_multiplier=-1,
            )
            nc.vector.tensor_copy(r2_sb[:], r2i_sb[:])
            vaug3 = vaug_sb[:].rearrange("p (k c) -> p k c", c=80)
            nc.vector.memset(vaug3[:, :, 64:65], 1.0)

            def emit_xtq_dma(h):
                # xtq is host-packed in slot processing order; half h=0
                # covers slots [3,2], h=1 covers [1,0]
                nc.gpsimd.dma_start(
                    out=xtqb_sb[:, h * 4096:(h + 1) * 4096],
                    in_=xtqb_d[:, h * 4096:(h + 1) * 4096],
                )

            emit_xtq_dma(0)

            with (
                tc.tile_pool(name="projps", bufs=2, space="PSUM") as projps,
                tc.tile_pool(name="stps", bufs=2, space="PSUM") as stps,
                tc.tile_pool(name="otps", bufs=1, space="PSUM") as otps,
                tc.tile_pool(name="ptp", bufs=8) as ptp,
                tc.tile_pool(name="epi", bufs=6) as epi,
                tc.tile_pool(name="ysbp", bufs=2) as ysbp,
            ):
                # PE HAM warm-up: dummy matmuls while input DMAs stream
                for _ in range(N_DUMMY):
                    dmy = projps.tile([128, 512], F32, name="pp", tag="pp")
                    nc.tensor.matmul(
                        dmy[:], lhsT=dummy_sb[:, 0:128], rhs=dummy_sb[:],
                        start=True, stop=True,
                    )

                kv_k = kvt_sb[64:128, :].rearrange(
                    "p (g u c) -> p g u c", u=2, c=128
                )

                def emit_KV(sb):
                    # KV projection for column block sb (k-tiles 4sb..4sb+3)
                    kvp = projps.tile([128, 512], F32, name="pp", tag="pp")
                    for j in range(4):
                        c0 = sb * 2048 + j * 512
                        nc.tensor.matmul(
                            kvp[:],
                            lhsT=wpack_sb[:, j * 128:(j + 1) * 128],
                            rhs=xtb_sb[:, c0:c0 + 512],
                            start=(j == 0),
                            stop=(j == 3),
                        )
                    nc.vector.tensor_scalar_add(
                        kvt_sb[:, sb * 512:(sb + 1) * 512], kvp[:],
                        b2t_sb[:, 0:1],
                    )

                def emit_VTPK(sb):
                    # repack K^T: even k-tiles -> partitions 0:64, odd -> 64:128
                    nc.gpsimd.dma_start(
                        out=ktp_sb[0:64, sb * 256:(sb + 1) * 256],
                        in_=kv_k[:, 2 * sb:2 * sb + 2, 0:1, :],
                    )
                    nc.gpsimd.dma_start(
                        out=ktp_sb[64:128, sb * 256:(sb + 1) * 256],
                        in_=kv_k[:, 2 * sb:2 * sb + 2, 1:2, :],
                    )
                    # V natural tiles via PE transpose
                    for kt in range(4 * sb, 4 * sb + 4):
                        vtp = projps.tile([128, 64], F16, name="pp", tag="pp")
                        nc.tensor.transpose(
                            vtp[:], kvt_sb[0:64, kt * 128:(kt + 1) * 128],
                            ident_sb[:],
                        )
                        nc.vector.tensor_copy(
                            vaug_sb[:, kt * 80:kt * 80 + 64], vtp[:]
                        )

                def emit_P(sb):
                    emit_KV(sb)
                    emit_VTPK(sb)

                def emit_Q(s):
                    hq, q2 = QPOS[s] // 2, QPOS[s] % 2
                    qp = projps.tile([128, 512], F32, name="pp", tag="pp")
                    for j in range(4):
                        c0 = hq * 4096 + j * 1024 + q2 * 512
                        nc.tensor.matmul(
                            qp[:],
                            lhsT=wpack_sb[:, 512 + j * 128:512 + (j + 1) * 128],
                            rhs=xtqb_sb[:, c0:c0 + 512],
                            start=(j == 0),
                            stop=(j == 3),
                        )
                    nc.vector.tensor_scalar_add(
                        qtp_sb[:, s * 512:(s + 1) * 512], qp[:],
                        b2t_sb[:, 1:2],
                    )

                groups = [(s, g) for s in SLOT_ORDER for g in range(NGRP[s])]
                otp_of = {}
                pt_of = {}

                def emit_S(i):
                    s, g = groups[i]
                    if g == 0:
                        otp_of[s] = (
                            otps.tile([H + 1, 512], F32, name="otpA", tag="otpA"),
                            otps.tile([H + 1, 512], F32, name="otpB", tag="otpB"),
                        )
                    masked = g >= NGRP[s] - 4
                    pe_mask = masked and s != 3
                    stp = stps.tile([128, 1024], F32, name="stp", tag="stp")
                    if s == 3:
                        # first slot: skip the K^T repack dependency, read
                        # K^T straight from kvt (both tiles on row group 64)
                        for u in range(2):
                            kt = 2 * g + u
                            nc.tensor.matmul(
                                stp[:, u * 512:(u + 1) * 512],
                                lhsT=kvt_sb[64:128, kt * 128:(kt + 1) * 128],
                                rhs=qtp_sb[64:128, s * 512:(s + 1) * 512],
                                start=True, stop=True,
                                tile_position=(64, 0),
                            )
                    else:
                        nc.tensor.matmul(
                            stp[:, 0:512],
                            lhsT=ktp_sb[0:64, g * 128:(g + 1) * 128],
                            rhs=qtp_sb[0:64, s * 512:(s + 1) * 512],
                            start=True, stop=not pe_mask,
                            tile_position=(0, 0),
                        )
                        nc.tensor.matmul(
                            stp[:, 512:1024],
                            lhsT=ktp_sb[64:128, g * 128:(g + 1) * 128],
                            rhs=qtp_sb[64:128, s * 512:(s + 1) * 512],
                            start=True, stop=not pe_mask,
                            tile_position=(64, 0),
                        )
                    if pe_mask:
                        # add 0/-2048 bias tiles into the scores on the PE
                        idx = s * 4 + (g - (NGRP[s] - 4))
                        for u in range(2):
                            nc.tensor.matmul(
                                stp[:, u * 512:(u + 1) * 512],
                                lhsT=id128_sb[:],
                                rhs=biasm_sb[:, idx * 1024 + u * 512:
                                             idx * 1024 + (u + 1) * 512],
                                start=False, stop=True,
                            )
                    pt = ptp.tile([128, 1024], F16, name="pt", tag="pt")
                    nc.scalar.activation(
                        pt[:], stp[:], mybir.ActivationFunctionType.Exp,
                        scale=1.0 / 64.0,
                    )
                    if masked and s == 3:
                        # slot 3 (pipeline-fill phase): mask P on the DVE
                        nc.vector.scalar_tensor_tensor(
                            pt[:], r2_sb[:], b2t_sb[:, 14 + g:15 + g], pt[:],
                            op0=mybir.AluOpType.is_ge,
                            op1=mybir.AluOpType.mult,
                        )
                    pt_of[i] = pt

                def emit_AV(i):
                    # even k-tiles accumulate into otpA, odd into otpB, so
                    # consecutive matmuls hit different PSUM banks and their
                    # fill/drain phases overlap
                    s, g = groups[i]
                    pt = pt_of.pop(i)
                    for u in range(2):
                        kt = 2 * g + u
                        nc.tensor.matmul(
                            otp_of[s][u][:],
                            lhsT=vaug_sb[:, kt * 80:kt * 80 + 65],
                            rhs=pt[:, u * 512:(u + 1) * 512],
                            start=(kt == u),
                            stop=(kt == NKT[s] - 2 + u),
                        )

                epi_st = {}

                def emit_E_half(s, half):
                    last = s == SLOT_ORDER[-1]
                    if half == 0:
                        otpA, otpB = otp_of.pop(s)
                        ot16 = epi.tile([H + 1, 512], F16, name="ot16", tag="ot16")
                        dnrow = epi.tile([1, 512], F16, name="dnrow", tag="dnrow")
                        nc.vector.tensor_copy(ot16[:], otpA[:])
                        nc.vector.tensor_add(ot16[:], ot16[:], otpB[:])
                        nc.vector.tensor_copy(dnrow[:], ot16[64:65, :])
                        ysbw = ysbp.tile([128, 2048], F16, name="ysbw", tag="ysbw")
                        # all four denominator transposes into one PSUM tile,
                        # one batched reciprocal
                        dnp = projps.tile([128, 4], F32, name="pp", tag="pp")
                        for t in range(4):
                            nc.tensor.matmul(
                                dnp[:, t:t + 1],
                                lhsT=dnrow[:, t * 128:(t + 1) * 128],
                                rhs=ones_sb[:],
                                start=True, stop=True,
                            )
                        recip = epi.tile([128, 4], F32, name="recip", tag="recip")
                        nc.vector.reciprocal(recip[:], dnp[:])
                        epi_st[s] = (ot16, dnrow, ysbw, recip)
                    ot16, dnrow, ysbw, recip = epi_st[s]
                    for t in (0, 1) if half == 0 else (2, 3):
                        yp = projps.tile([128, 512], F32, name="pp", tag="pp")
                        nc.tensor.matmul(
                            yp[:],
                            lhsT=ot16[:, t * 128:(t + 1) * 128],
                            rhs=wpack_sb[0:65, 1024:1536],
                            start=True, stop=True,
                        )
                        if last and t in (1, 3):
                            # scalar engine is done with exps by now: use its
                            # free affine to scale, halving the tail chain
                            nc.scalar.activation(
                                ysbw[:, t * 512:(t + 1) * 512], yp[:],
                                mybir.ActivationFunctionType.Copy,
                                scale=recip[:, t:t + 1],
                            )
                        else:
                            nc.vector.tensor_scalar_mul(
                                ysbw[:, t * 512:(t + 1) * 512], yp[:],
                                recip[:, t:t + 1],
                            )
                        if last and t in (1, 3):
                            nc.sync.dma_start(
                                out=out_d[:, s * 2048 + (t - 1) * 512:
                                          s * 2048 + (t + 1) * 512],
                                in_=ysbw[:, (t - 1) * 512:(t + 1) * 512],
                            )
                    if half == 1 and not last:
                        nc.sync.dma_start(
                            out=out_d[:, s * 2048:(s + 1) * 2048],
                            in_=ysbw[:],
                        )

                # ---- software-pipelined emission ----
                prod = {
                    0: [lambda: emit_P(0), lambda: emit_Q(3)],
                    2: [lambda: emit_P(1), lambda: emit_Q(2),
                        lambda: emit_xtq_dma(1)],
                    4: [lambda: emit_P(2)],
                    8: [lambda: emit_P(3)],
                    12: [lambda: emit_Q(1), lambda: emit_P(4)],
                    16: [lambda: emit_P(5)],
                    20: [lambda: emit_P(6)],
                    24: [lambda: emit_Q(0)],
                    28: [lambda: emit_P(7)],
                }
                last_step_of_slot = {}
                acc = -1
                for s in SLOT_ORDER:
                    acc += NGRP[s]
                    last_step_of_slot[s] = acc

                n = len(groups)
                for i in range(n + 5):
                    for fn in prod.get(i, []):
                        fn()
                    if i < n:
                        emit_S(i)
                    if 0 <= i - 4 < n:
                        emit_AV(i - 4)
                        for s in SLOT_ORDER:
                            if last_step_of_slot[s] == i - 4:
                                emit_E_half(s, 0)
                    if 0 <= i - 5 < n:
                        for s in SLOT_ORDER:
                            if last_step_of_slot[s] == i - 5:
                                emit_E_half(s, 1)

    nc.compile()
    return nc


_NC_CACHE = {}


def _thr_row(blocks):
    # mask P[k_local, u*512+f] iff  f - p - 128*u < thr[s, j]
    # thr = 128*t0 - 512*block  with t0 = NKT[s]-8+2j  (even tile of group)
    t = np.zeros(16, np.float32)
    for s in range(4):
        for j in range(4):
            t0 = NKT[s] - 8 + 2 * j
            t[s * 4 + j] = 128.0 * t0 - 512.0 * blocks[s]
    return t


def _bias_tiles(blocks):
    # bias[p, idx*1024 + u*512+f] = NEG_BIAS where masked (slots 0-2 only)
    p = np.arange(128)[:, None, None]
    cols = np.arange(1024)[None, None, :]
    r2 = (cols % 512) - p - 128 * (cols // 512)
    thr = _thr_row(blocks)[:12].reshape(1, 12, 1)
    bias = np.where(r2 < thr, np.float32(NEG_BIAS), np.float32(0.0))
    return bias.reshape(128, 12 * 1024).astype(np.float16)


def _make_in_maps(x, Wq, bq, Wk, bk, Wv, bv, Wo, bo):
    # weight pack: [wkv(j-chunked) | wq2(j-chunked) | wo_aug padded]
    wkv = np.concatenate([Wv, Wk], axis=1).astype(np.float16)
    wkv = wkv.reshape(4, 128, 128).transpose(1, 0, 2).reshape(128, 512)
    wq2 = np.concatenate([Wq, Wq], axis=1).astype(np.float16)
    wq2 = wq2.reshape(4, 128, 128).transpose(1, 0, 2).reshape(128, 512)
    wo_aug = np.concatenate([Wo, (bv @ Wo + bo)[None, :]], axis=0)
    wop = np.zeros((128, 512), np.float16)
    wop[0:65] = wo_aug.astype(np.float16)
    wpack = np.ascontiguousarray(
        np.concatenate([wkv, wq2, wop], axis=1))           # (128, 1536)

    # biases + slot-3 thresholds: [bkv | bq2 | thr(16)]
    bkv = np.concatenate([np.zeros(64, np.float32), bk])
    bq2 = np.concatenate([bq, bq])
    thr_e, thr_o = _thr_row(BLOCKS_EVEN), _thr_row(BLOCKS_ODD)
    b2t_e = np.concatenate(
        [bkv[:, None], bq2[:, None], np.tile(thr_e[None, :], (128, 1))],
        axis=1).astype(np.float32)
    b2t_o = np.concatenate(
        [bkv[:, None], bq2[:, None], np.tile(thr_o[None, :], (128, 1))],
        axis=1).astype(np.float32)

    biasm_even = _bias_tiles(BLOCKS_EVEN)
    biasm_odd = _bias_tiles(BLOCKS_ODD)

    in_maps = []
    for c in range(8):
        b = c // 2
        blocks = BLOCKS_EVEN if c % 2 == 0 else BLOCKS_ODD
        xt = np.ascontiguousarray(x[b].T).astype(np.float16)      # (512, 4096)
        # xtb[p, sb*2048 + j*512 + cc] = xt[j*128+p, sb*512+cc]
        xtb = np.ascontiguousarray(
            xt.reshape(4, 128, 8, 512).transpose(1, 2, 0, 3).reshape(128, 16384))
        qcols = np.concatenate(
            [np.arange(blocks[s] * QB, (blocks[s] + 1) * QB) for s in SLOT_ORDER]
        )
        xtq = xt[:, qcols]                                        # (512, 2048)
        # xtqb[p, H*4096 + j*1024 + q*512 + cc] = xtq[j*128+p, (2H+q)*512+cc]
        xtqb = np.ascontiguousarray(
            xtq.reshape(4, 128, 2, 2, 512).transpose(1, 2, 0, 3, 4).reshape(128, 8192))
        in_maps.append({
            "xtb": xtb,
            "xtqb": xtqb,
            "wpack": wpack,
            "b2t": b2t_e if c % 2 == 0 else b2t_o,
            "biasm": biasm_even if c % 2 == 0 else biasm_odd,
        })
    return in_maps


def kernel(x, Wq, bq, Wk, bk, Wv, bv, Wo, bo):
    global LAST_EXEC_TIME_NS, LAST_RESULTS
    x = np.asarray(x, dtype=np.float32)
    Wq, bq = np.asarray(Wq, np.float32), np.asarray(bq, np.float32)
    Wk, bk = np.asarray(Wk, np.float32), np.asarray(bk, np.float32)
    Wv, bv = np.asarray(Wv, np.float32), np.asarray(bv, np.float32)
    Wo, bo = np.asarray(Wo, np.float32), np.asarray(bo, np.float32)

    if "nc" not in _NC_CACHE:
        _NC_CACHE["nc"] = _build_nc()
    nc = _NC_CACHE["nc"]

    in_maps = _make_in_maps(x, Wq, bq, Wk, bk, Wv, bv, Wo, bo)

    trace = os.environ.get("KERNEL_TRACE", "1") == "1"
    if trace:
        trace = _install_ntff_hook()
    tmpdir = os.environ.get("KERNEL_TRACE_DIR") or None
    try:
        res = run_bass_kernel_spmd(
            nc, in_maps, core_ids=list(range(8)), trace=trace, tmpdir=tmpdir
        )
    except Exception:
        if not trace:
            raise
        res = run_bass_kernel_spmd(nc, in_maps, core_ids=list(range(8)), trace=False)
    LAST_EXEC_TIME_NS = res.exec_time_ns
    LAST_RESULTS = res

    out = np.empty((B, S, D), np.float32)
    for c in range(8):
        b = c // 2
        blocks = BLOCKS_EVEN if c % 2 == 0 else BLOCKS_ODD
        shard2 = np.asarray(res.results[c]["out"], dtype=np.float32)
        # shard2[p, s*2048 + t*512 + cc] = y[slot s][t*128+p, cc]
        y = shard2.reshape(128, 4, 4, 512).transpose(1, 2, 0, 3).reshape(4, 512, 512)
        for s in range(4):
            out[b, blocks[s] * QB:(blocks[s] + 1) * QB, :] = y[s]
    return out
